# revision 40
# baseline (speedup 1.0000x reference)
"""DiffNet GNN message-passing kernel for 8 Trainium2 NeuronCores.

Math: final_user = t2/deg_soc + 2*h1 + t3/deg_info, restricted to batch users,
where h1 = A_soc@u0/deg_soc + u0 (needed for ALL users since layer 2 gathers
arbitrary columns), t2 = A_soc@h1 at batch rows only, t3 = A_info@item_emb at
batch rows only. Output = sigmoid(2 * sum(final_user[uids] * item_emb[iids])).

Sharding: by output row range (12500 users/core). Layer-1 SpMM over the full
edge set partitions exactly by row; one chunked AllGather publishes h1; layer-2
and info SpMMs run only on each core's batch-user rows.

Device SpMM: edges sorted by (group, col-chunk, tile, col); per-edge message
rows pulled from HBM by SWDGE dma_gather round-robined over 4 SWDGE queues
(each queue runs on its own GPSIMD DSP core pair, so descriptor generation for
4 gathers proceeds concurrently). user/item tables are bf16 padded to a 256B
row stride so each gather descriptor moves only 128B. Segment-sum: a one-hot
matrix (batched DVE is_equal) is the PE stationary [128e,128r], messages move
[128e,64d], accumulating row-major [128r,64d] tiles in PSUM — no transposes.
"""

import sys

sys.path.insert(0, "/opt/trn_rl_repo")

import math
import os

import numpy as np
import ml_dtypes

import concourse.bacc as bacc
import concourse.bass as bass
import concourse.mybir as mybir
import concourse.tile as tile
from concourse.bass_utils import run_bass_kernel_spmd

P = 128
BF16 = ml_dtypes.bfloat16
GSUB = 4096  # max idxs per dma_gather sub-call (split across SWDGE queues)


class Cfg:
    def __init__(self, n_user, n_item, d, n_cores, tpg1, gpa, tpg2, chunk):
        self.n_user = n_user
        self.n_item = n_item
        self.d = d
        self.nc = n_cores
        self.rpc = n_user // n_cores  # rows per core
        t1 = -(-self.rpc // P)  # L1 tiles per core (unpadded)
        self.tpg1 = tpg1  # L1 tiles per group
        self.t1p = -(-t1 // tpg1) * tpg1  # padded L1 tile count
        self.ng1 = self.t1p // tpg1
        self.gpa = gpa  # groups per AllGather chunk
        assert self.ng1 % gpa == 0
        self.agc = self.ng1 // gpa  # number of AG chunks
        self.cr = tpg1 * P * gpa  # rows per core per AG chunk
        self.shard_rows = self.t1p * P
        self.flat_h1 = self.nc * self.shard_rows  # h1_full rows
        self.tpg2 = tpg2  # batch tiles per group (L2 & info)
        self.chunk = chunk  # max gather-chunk rows (int16 limit)
        self.nch_u = -(-n_user // chunk)
        self.ch_u = -(-n_user // self.nch_u)
        self.nch_i = -(-n_item // chunk)
        self.ch_i = -(-n_item // self.nch_i)
        self.nch_h = -(-self.flat_h1 // chunk)
        self.ch_h = -(-self.flat_h1 // self.nch_h)


REAL = Cfg(100000, 50000, 64, 8, 7, 2, 4, 25088)


def _wrap_idx(idx_call):
    """[n] int16 -> [128, n/16] wrapped+replicated."""
    n = idx_call.shape[0]
    a = idx_call.reshape(n // 16, 16).T  # [16, n/16]
    return np.tile(a, (8, 1))


def _gather_raw(nc, out_ap, in_ap, idxs_ap, num_idxs, elem_size, elem_step,
                queue_num):
    """dma_gather without the 256B-payload restriction.

    in_ap rows live at a 256B stride (elem_step elements) but only elem_size
    elements (128B for bf16 d=64) are moved per descriptor.
    """
    g = nc.gpsimd
    stride_bytes = elem_step * mybir.dt.size(in_ap.dtype)
    assert stride_bytes % 256 == 0
    _in_ap = g.lower_ap_dma(in_ap, for_custom_bir_dma=True)
    _idxs_ap = g.lower_ap(idxs_ap)
    _out_ap = g.lower_ap(out_ap)
    return g.add_instruction(
        mybir.InstDMAGatherAnt(
            name=g.bass.get_next_instruction_name(),
            ins=[*_in_ap, _idxs_ap, g.lower_val_access(g.to_reg(num_idxs))],
            outs=[_out_ap],
            transpose=False,
            num_idxs=num_idxs,
            elem_size=elem_size,
            stride_bytes_256=stride_bytes // 256,
            gen_mode=0,
            single_packet=False,
            queue_num=queue_num,
            sbuf_tokens_per_rank=0,
            sbuf_free_dim_per_rank=0,
            sbuf_free_dim_pad_per_rank=0,
            sbuf_byte_offset=0,
        )
    )


class SpmmSched:
    """SPMD-uniform slot/block layout for one SpMM (same across cores)."""

    def __init__(self, ntp, tpg, nch):
        self.ntp = ntp  # padded tile count
        self.tpg = tpg
        self.ng = ntp // tpg
        self.nch = nch
        self.cap = None  # [ntp, nch] slots, multiples of 128

    def finalize(self):
        ntp, tpg, ng, nch = self.ntp, self.tpg, self.ng, self.nch
        cap = self.cap
        # ensure every tile has >=1 block so its PSUM region gets zeroed
        for t in range(ntp):
            if cap[t].sum() == 0:
                cap[t, 0] = P
        # region = (g, c): tiles g*tpg..g*tpg+tpg-1
        self.sub_off = np.zeros((ntp, nch), np.int64)  # slot offset in region
        self.region_nidx = np.zeros((ng, nch), np.int64)
        self.slot_base = np.zeros((ng, nch), np.int64)  # global slot offset
        self.blk_base = np.zeros((ng, nch), np.int64)
        self.group_blk0 = np.zeros(ng, np.int64)
        s = 0
        b = 0
        for g in range(ng):
            self.group_blk0[g] = b
            for c in range(nch):
                self.slot_base[g, c] = s
                self.blk_base[g, c] = b
                off = 0
                for tl in range(tpg):
                    t = g * tpg + tl
                    self.sub_off[t, c] = off
                    off += cap[t, c]
                self.region_nidx[g, c] = off
                s += off
                b += off // P
        self.total_slots = s
        self.total_blocks = b
        self.group_blocks = [
            int(sum(self.region_nidx[g]) // P) for g in range(ng)
        ]
        # per (g, tl): ordered list of global block ids (for start/stop flags)
        self.tile_blocks = {}
        for g in range(ng):
            for tl in range(self.tpg):
                t = g * self.tpg + tl
                blks = []
                for c in range(nch):
                    b0 = self.blk_base[g, c] + self.sub_off[t, c] // P
                    blks += list(range(b0, b0 + cap[t, c] // P))
                self.tile_blocks[(g, tl)] = blks
        # idx array column offsets (global, in units of 16 slots)
        self.idx_off = np.zeros((ng, nch), np.int64)
        w = 0
        for g in range(ng):
            for c in range(nch):
                self.idx_off[g, c] = w
                w += self.region_nidx[g, c] // 16
        self.idx_w = w


def _sched_caps(sched, per_core_tc_counts):
    """per_core_tc_counts: list of [ntp, nch] arrays -> set caps."""
    mx = np.maximum.reduce(per_core_tc_counts)
    sched.cap = (-(-mx // P) * P).astype(np.int64)
    sched.finalize()


def _fill_spmm(sched, rows_t, cols_c, col_idx, rowloc, vals):
    """Place one core's edges into the schedule's slot space.

    rows_t: tile id per edge; cols_c: chunk id; col_idx: int16 local col;
    rowloc: row-in-tile (0..127); vals: edge values (float32).
    Edges are sorted by column within each (tile, chunk) so the gather's HBM
    addresses ascend (DRAM locality).
    Returns (idx_arr [128, idx_w] i16, rl [128, B] bf16, val_w [128, B] f32,
             counts [ng*nch] i32).
    """
    ntp, tpg, ng, nch = sched.ntp, sched.tpg, sched.ng, sched.nch
    g_e = rows_t // tpg
    tl_e = rows_t % tpg
    bid = (g_e * nch + cols_c) * tpg + tl_e
    order = np.lexsort((col_idx, bid))
    bid_s = bid[order]
    counts = np.bincount(bid_s, minlength=ng * nch * tpg)
    starts = np.concatenate([[0], np.cumsum(counts)[:-1]])
    rank = np.arange(len(bid_s)) - starts[bid_s]
    t_s = rows_t[order]
    c_s = cols_c[order]
    g_s = g_e[order]
    slot = (
        sched.slot_base[g_s, c_s]
        + sched.sub_off[t_s, c_s]
        + rank
    )
    ns = sched.total_slots
    idx_flat = np.zeros(ns, np.int32)
    rl_flat = np.full(ns, -1.0, np.float32)
    val_flat = np.zeros(ns, np.float32)
    idx_flat[slot] = col_idx[order]
    rl_flat[slot] = rowloc[order]
    val_flat[slot] = vals[order]
    # pad slots keep idx 0 (real harmless gathers) so every slot is always
    # written -- avoids NaN garbage flowing into the matmul.
    call_counts = np.zeros(ng * nch, np.int32)
    # wrap
    idx_arr = np.empty((P, sched.idx_w), np.int16)
    for g in range(ng):
        for c in range(nch):
            n = sched.region_nidx[g, c]
            if n == 0:
                continue
            s0 = sched.slot_base[g, c]
            w0 = sched.idx_off[g, c]
            idx_arr[:, w0 : w0 + n // 16] = _wrap_idx(
                idx_flat[s0 : s0 + n].astype(np.int16)
            )
    # host-precomputed one-hot routing matrices, fp8 (exact for 0/1):
    # oh[e, b*128 + r] = 1 iff edge in slot (b, e) targets row r of its tile.
    # Streamed from HBM instead of generated on DVE (is_equal runs at the
    # errata-limited 1x rate and was a ~1ms bottleneck).
    FP8 = mybir.dt.np(mybir.dt.float8e4)
    rl_i = rl_flat.reshape(sched.total_blocks, P).T.astype(np.int32)  # [P, B]
    oh = (rl_i[:, :, None] == np.arange(P, dtype=np.int32)[None, None, :])
    oh = np.ascontiguousarray(oh.astype(FP8).reshape(P, sched.total_blocks * P))
    val_w = np.ascontiguousarray(val_flat.reshape(sched.total_blocks, P).T)
    return idx_arr, oh, val_w, call_counts


def _prep(cfg, inputs):
    """All host-side preprocessing. Returns (plan, in_maps, out_meta)."""
    nc_, d = cfg.nc, cfg.d
    user_emb = np.asarray(inputs["user_emb"], np.float32)
    item_emb = np.asarray(inputs["item_emb"], np.float32)
    s_rows = np.asarray(inputs["social_rows"], np.int64)
    s_cols = np.asarray(inputs["social_cols"], np.int64)
    s_vals = np.asarray(inputs["social_vals"], np.float32)
    i_rows = np.asarray(inputs["info_rows"], np.int64)
    i_cols = np.asarray(inputs["info_cols"], np.int64)
    i_vals = np.asarray(inputs["info_vals"], np.float32)
    uids = np.asarray(inputs["user_ids"], np.int64)
    iids = np.asarray(inputs["item_ids"], np.int64)
    eps = 1e-8

    ones = bool(np.all(s_vals == 1.0) and np.all(i_vals == 1.0))

    deg_soc = np.bincount(s_rows, weights=s_vals, minlength=cfg.n_user)
    deg_info = np.bincount(i_rows, weights=i_vals, minlength=cfg.n_user)
    inv_soc = (1.0 / (deg_soc.astype(np.float32) + eps)).astype(np.float32)
    inv_info = (1.0 / (deg_info.astype(np.float32) + eps)).astype(np.float32)

    # padded tables: 256B row stride, payload in cols [:64].
    # user table is fp8e4 (64B payloads halve gather packet bytes); values are
    # scaled by 64 to clear the fp8 subnormal range, compensated via ist/64.
    FP8 = mybir.dt.np(mybir.dt.float8e4)
    user_pad = np.zeros((cfg.n_user, 4 * d), FP8)
    user_pad[:, :d] = (user_emb * 64.0).astype(FP8)
    item_pad = np.zeros((cfg.n_item, 2 * d), BF16)
    item_pad[:, :d] = item_emb

    # batch users
    uniq = np.unique(uids)
    owner = uniq // cfg.rpc
    bu = [uniq[owner == c] for c in range(nc_)]
    ubmax = max(len(b) for b in bu)
    ubt = -(-ubmax // P)
    ng2 = max(1, -(-ubt // cfg.tpg2))
    ubt_p = ng2 * cfg.tpg2
    ubp = ubt_p * P

    # --- L1 schedule ---
    s1 = SpmmSched(cfg.t1p, cfg.tpg1, cfg.nch_u)
    order = np.argsort(s_rows, kind="stable")
    sr, sc, sv = s_rows[order], s_cols[order], s_vals[order]
    bounds = np.searchsorted(sr, [c * cfg.rpc for c in range(nc_ + 1)])
    core_l1 = []
    tc_counts = []
    for c in range(nc_):
        lo, hi = bounds[c], bounds[c + 1]
        lr = sr[lo:hi] - c * cfg.rpc
        col = sc[lo:hi]
        t = lr // P
        ch = col // cfg.ch_u
        core_l1.append((t, ch, (col - ch * cfg.ch_u), lr % P, sv[lo:hi]))
        m = np.zeros((cfg.t1p, cfg.nch_u), np.int64)
        np.add.at(m, (t, ch), 1)
        tc_counts.append(m)
    _sched_caps(s1, tc_counts)

    # --- L2 & info: batch-row-restricted ---
    slot_of = np.full(cfg.n_user, -1, np.int64)
    for c in range(nc_):
        slot_of[bu[c]] = np.arange(len(bu[c]))
    in_batch = slot_of >= 0

    def batch_edges(rows, cols, vals):
        m = in_batch[rows]
        r, co, v = rows[m], cols[m], vals[m]
        core = r // cfg.rpc
        return r, co, v, core

    s2 = SpmmSched(ubt_p, cfg.tpg2, cfg.nch_h)
    si = SpmmSched(ubt_p, cfg.tpg2, cfg.nch_i)

    def h1_flat(col):
        own = col // cfg.rpc
        lr = col - own * cfg.rpc
        k = lr // cfg.cr
        off = lr - k * cfg.cr
        return k * (nc_ * cfg.cr) + own * cfg.cr + off

    r2, c2, v2, core2 = batch_edges(sr, sc, sv)
    f2 = h1_flat(c2)
    ri, ci, vi, corei = batch_edges(i_rows, i_cols, i_vals)

    core_l2, core_in = [], []
    tc2, tci = [], []
    for c in range(nc_):
        m = core2 == c
        sl = slot_of[r2[m]]
        t = sl // P
        ch = f2[m] // cfg.ch_h
        core_l2.append((t, ch, f2[m] - ch * cfg.ch_h, sl % P, v2[m]))
        a = np.zeros((ubt_p, cfg.nch_h), np.int64)
        np.add.at(a, (t, ch), 1)
        tc2.append(a)
        m = corei == c
        sl = slot_of[ri[m]]
        t = sl // P
        ch = ci[m] // cfg.ch_i
        core_in.append((t, ch, ci[m] - ch * cfg.ch_i, sl % P, vi[m]))
        a = np.zeros((ubt_p, cfg.nch_i), np.int64)
        np.add.at(a, (t, ch), 1)
        tci.append(a)
    _sched_caps(s2, tc2)
    _sched_caps(si, tci)

    # --- final pairs ---
    pcore = uids // cfg.rpc
    pch = iids // cfg.ch_i
    fcap = np.zeros(cfg.nch_i, np.int64)
    per_core_pairs = []
    for c in range(nc_):
        m = np.nonzero(pcore == c)[0]
        o = m[np.lexsort((iids[m], pch[m]))]
        per_core_pairs.append(o)
        cnts = np.bincount(pch[o], minlength=cfg.nch_i)
        fcap = np.maximum(fcap, cnts)
    fcap = -(-fcap // P) * P
    fcap = np.maximum(fcap, P)
    fbase = np.concatenate([[0], np.cumsum(fcap)])
    ftot = int(fbase[-1])

    plan = dict(
        cfg=cfg, s1=s1, s2=s2, si=si, ubt_p=ubt_p, ubp=ubp, ng2=ng2,
        fcap=fcap, fbase=fbase, ftot=ftot, ones=ones,
    )

    in_maps = []
    out_meta = []  # per core: (pair_js, slots)
    for c in range(nc_):
        t, ch, cidx, rl, v = core_l1[c]
        l1_idx, l1_oh, l1_val, cnt1 = _fill_spmm(s1, t, ch, cidx, rl, v)
        t, ch, cidx, rl, v = core_l2[c]
        l2_idx, l2_oh, l2_val, cnt2 = _fill_spmm(s2, t, ch, cidx, rl, v)
        t, ch, cidx, rl, v = core_in[c]
        in_idx, in_oh, in_val, cnti = _fill_spmm(si, t, ch, cidx, rl, v)

        # u0 shard
        u0s = np.zeros((cfg.shard_rows, d), np.float32)
        nrow = min(cfg.rpc, cfg.n_user - c * cfg.rpc)
        u0s[:nrow] = user_emb[c * cfg.rpc : c * cfg.rpc + nrow]

        # invdeg arrays
        # /64 compensates the fp8 user-table scaling baked into L1 psums
        ist = np.zeros((P, cfg.t1p), np.float32)
        rows = c * cfg.rpc + np.arange(nrow)
        ist[np.arange(nrow) % P, np.arange(nrow) // P] = inv_soc[rows] / 64.0
        isb = np.zeros((P, ubt_p), np.float32)
        iib = np.zeros((P, ubt_p), np.float32)
        nb = len(bu[c])
        isb[np.arange(nb) % P, np.arange(nb) // P] = inv_soc[bu[c]]
        iib[np.arange(nb) % P, np.arange(nb) // P] = inv_info[bu[c]]

        # h1 batch gather idx (local shard rows); pads gather row 0
        h1b = np.zeros(ubp, np.int16)
        h1b[:nb] = (bu[c] - c * cfg.rpc).astype(np.int16)

        # final pairs
        o = per_core_pairs[c]
        pu = np.zeros(ftot, np.int16)
        pi = np.zeros(ftot, np.int16)
        slots = np.empty(len(o), np.int64)
        pos = 0
        for chn in range(cfg.nch_i):
            sel = o[pch[o] == chn]
            k = len(sel)
            s0 = fbase[chn]
            pu[s0 : s0 + k] = slot_of[uids[sel]].astype(np.int16)
            pi[s0 : s0 + k] = (iids[sel] - chn * cfg.ch_i).astype(np.int16)
            slots[pos : pos + k] = s0 + np.arange(k)
            pos += k
        out_meta.append((o, slots))

        m = {
            "user_pad": user_pad,
            "item_pad": item_pad,
            "item_emb": item_emb,
            "u0s": u0s,
            "l1_idx": l1_idx, "l1_oh": l1_oh,
            "l2_idx": l2_idx, "l2_oh": l2_oh,
            "in_idx": in_idx, "in_oh": in_oh,
            "ist": ist, "isb": isb, "iib": iib,
            "h1b_idx": _wrap_idx(h1b),
            "pu_idx": _wrap_idx(pu), "pi_idx": _wrap_idx(pi),
        }
        if not ones:
            m["l1_val"] = l1_val
            m["l2_val"] = l2_val
            m["in_val"] = in_val
        in_maps.append(m)
    return plan, in_maps, out_meta


def _build_program(plan):
    cfg = plan["cfg"]
    s1, s2, si = plan["s1"], plan["s2"], plan["si"]
    ubt_p, ubp, ng2 = plan["ubt_p"], plan["ubp"], plan["ng2"]
    fcap, fbase, ftot = plan["fcap"], plan["fbase"], plan["ftot"]
    ones = plan["ones"]
    d = cfg.d
    nc_ = cfg.nc
    f32 = mybir.dt.float32
    bf = mybir.dt.bfloat16

    # 4 SWDGE queues: each runs on its own GPSIMD DSP core pair, so gathers
    # on different queues generate descriptors concurrently.
    nc = bacc.Bacc("TRN2", debug=False, num_devices=nc_, num_swdge_queues=4)
    qrr = [0]

    def next_q():
        qrr[0] = (qrr[0] + 1) % 4
        return qrr[0]

    t_user = nc.dram_tensor("user_pad", [cfg.n_user, 4 * d], mybir.dt.float8e4, kind="ExternalInput")
    t_itemp = nc.dram_tensor("item_pad", [cfg.n_item, 2 * d], bf, kind="ExternalInput")
    t_item = nc.dram_tensor("item_emb", [cfg.n_item, d], f32, kind="ExternalInput")
    t_u0s = nc.dram_tensor("u0s", [cfg.shard_rows, d], f32, kind="ExternalInput")
    fp8 = mybir.dt.float8e4
    t_l1i = nc.dram_tensor("l1_idx", [P, s1.idx_w], mybir.dt.int16, kind="ExternalInput")
    t_l1o = nc.dram_tensor("l1_oh", [P, s1.total_blocks * P], fp8, kind="ExternalInput")
    t_l2i = nc.dram_tensor("l2_idx", [P, s2.idx_w], mybir.dt.int16, kind="ExternalInput")
    t_l2o = nc.dram_tensor("l2_oh", [P, s2.total_blocks * P], fp8, kind="ExternalInput")
    t_ini = nc.dram_tensor("in_idx", [P, si.idx_w], mybir.dt.int16, kind="ExternalInput")
    t_ino = nc.dram_tensor("in_oh", [P, si.total_blocks * P], fp8, kind="ExternalInput")
    t_ist = nc.dram_tensor("ist", [P, cfg.t1p], f32, kind="ExternalInput")
    t_isb = nc.dram_tensor("isb", [P, ubt_p], f32, kind="ExternalInput")
    t_iib = nc.dram_tensor("iib", [P, ubt_p], f32, kind="ExternalInput")
    t_h1bi = nc.dram_tensor("h1b_idx", [P, ubp // 16], mybir.dt.int16, kind="ExternalInput")
    t_pui = nc.dram_tensor("pu_idx", [P, ftot // 16], mybir.dt.int16, kind="ExternalInput")
    t_pii = nc.dram_tensor("pi_idx", [P, ftot // 16], mybir.dt.int16, kind="ExternalInput")
    t_scores = nc.dram_tensor("scores", [P, ftot // P], f32, kind="ExternalOutput")
    t_vals = {}
    if not ones:
        t_vals["l1"] = nc.dram_tensor("l1_val", [P, s1.total_blocks], f32, kind="ExternalInput")
        t_vals["l2"] = nc.dram_tensor("l2_val", [P, s2.total_blocks], f32, kind="ExternalInput")
        t_vals["in"] = nc.dram_tensor("in_val", [P, si.total_blocks], f32, kind="ExternalInput")

    with tile.TileContext(nc) as tc:
        with (
            tc.tile_pool(name="const", bufs=1) as cp,
            tc.tile_pool(name="persist", bufs=1) as pp,
            tc.tile_pool(name="idx", bufs=10) as idxp,
            tc.tile_pool(name="msgs", bufs=8) as msgp,
            tc.tile_pool(name="msgf", bufs=2) as mfp,
            tc.tile_pool(name="oh", bufs=6) as ohp,
            tc.tile_pool(name="rl", bufs=6) as rlp,
            tc.tile_pool(name="u0t", bufs=2) as u0p,
            tc.tile_pool(name="hrow", bufs=4) as hp,
            tc.tile_pool(name="psacc", bufs=4, space="PSUM") as pap,
            tc.tile_pool(name="psinfo", bufs=2, space="PSUM") as ipap,
            tc.tile_pool(name="psl2", bufs=2, space="PSUM") as l2ap,
            tc.tile_pool(name="dram", bufs=1, space="DRAM") as dram,
        ):
            # ---- constants / persistent ----
            ist_t = pp.tile([P, cfg.t1p], f32, tag="ist")
            nc.sync.dma_start(ist_t[:], t_ist.ap())
            isb_t = pp.tile([P, ubt_p], f32, tag="isb")
            nc.sync.dma_start(isb_t[:], t_isb.ap())
            iib_t = pp.tile([P, ubt_p], f32, tag="iib")
            nc.sync.dma_start(iib_t[:], t_iib.ap())
            t3R = pp.tile([P, ubt_p, d], f32, tag="t3R")
            h1b_t = pp.tile([P, ubt_p, d], f32, tag="h1b")
            nc.vector.memzero(h1b_t[:])

            # internal DRAM
            h1ag = [
                dram.tile([cfg.cr, d], bf, tag=f"h1ag{k}", name=f"h1ag{k}")
                for k in range(cfg.agc)
            ]
            h1fb = [
                dram.tile([nc_ * cfg.cr, d], bf, tag=f"h1fb{k}",
                          name=f"h1fb{k}")
                for k in range(cfg.agc)
            ]
            h1_full = dram.tile([cfg.flat_h1, 2 * d], bf, tag="h1full")
            h1_shard = dram.tile([cfg.shard_rows, d], f32, tag="h1shard")
            fu_tab = dram.tile([ubp, d], f32, tag="futab")

            def load_rl(sched, g, val_t):
                gb0 = int(sched.group_blk0[g])
                gblocks = sched.group_blocks[g]
                vw_t = None
                if val_t is not None:
                    vw_t = rlp.tile([P, gblocks], f32, tag="vw")
                    nc.sync.dma_start(vw_t[:], val_t.ap()[:, gb0 : gb0 + gblocks])
                first = {tl: sched.tile_blocks[(g, tl)][0] for tl in range(sched.tpg)
                         if sched.tile_blocks[(g, tl)]}
                last = {tl: sched.tile_blocks[(g, tl)][-1] for tl in range(sched.tpg)
                        if sched.tile_blocks[(g, tl)]}
                return vw_t, gb0, first, last

            def spmm_region(sched, g, c, grp, t_idx, t_oh, table_aps, psums,
                            bf_gather):
                """Emit gathers/onehot-load/matmuls for one (group, chunk)
                region.

                psums[tl]: PSUM [128, d] accumulator slice for each tile.
                bf_gather: gather 128B bf16 payloads from a padded table
                (table_aps are [:, :64] views of 256B-stride bf16 tables);
                else classic 256B f32 dma_gather.
                """
                vw_t, gb0, first, last = grp
                nidx = int(sched.region_nidx[g, c])
                if nidx == 0:
                    return
                rb = nidx // P
                w0 = int(sched.idx_off[g, c])
                it = idxp.tile([P, nidx // 16], mybir.dt.int16, tag="idx")
                nc.sync.dma_start(it[:], t_idx.ap()[:, w0 : w0 + nidx // 16])
                # prefetch the fp8 one-hot stream so it overlaps the gathers
                b0 = int(sched.blk_base[g, c])
                oh_t = ohp.tile([P, rb * P], mybir.dt.float8e4, tag="oh")
                nc.sync.dma_start(
                    oh_t[:], t_oh.ap()[:, b0 * P : (b0 + rb) * P]
                )
                if bf_gather:
                    gdt = table_aps[c].dtype
                    estep = (4 * d) if gdt == mybir.dt.float8e4 else (2 * d)
                    m_t = msgp.tile([P, rb, d], gdt, tag="msgs")
                    for s0 in range(0, nidx, GSUB):
                        n = min(GSUB, nidx - s0)
                        _gather_raw(
                            nc,
                            m_t[:, s0 // P : (s0 + n) // P, :],
                            table_aps[c],
                            it[:, s0 // 16 : (s0 + n) // 16],
                            n, d, estep, next_q(),
                        )
                else:
                    m_t = mfp.tile([P, rb, d], f32, tag="msgf")
                    for s0 in range(0, nidx, GSUB):
                        n = min(GSUB, nidx - s0)
                        nc.gpsimd.dma_gather(
                            m_t[:, s0 // P : (s0 + n) // P, :],
                            table_aps[c],
                            it[:, s0 // 16 : (s0 + n) // 16],
                            n, n, d, single_packet=False,
                            queue_num=next_q(),
                        )
                if vw_t is not None:
                    rboff0 = int(sched.blk_base[g, c]) - gb0
                    nc.vector.tensor_tensor(
                        out=m_t[:],
                        in0=m_t[:],
                        in1=vw_t[:, rboff0 : rboff0 + rb]
                        .unsqueeze(2)
                        .to_broadcast([P, rb, d]),
                        op=mybir.AluOpType.mult,
                    )
                mb_src = m_t
                if not bf_gather:
                    mb_t = msgp.tile([P, rb, d], bf, tag="msgs")
                    nc.scalar.copy(mb_t[:], m_t[:])
                    mb_src = mb_t
                for j in range(rb):
                    gblk = b0 + j
                    soff = j * P
                    tl = 0
                    for tt in range(sched.tpg):
                        t_ = g * sched.tpg + tt
                        if (sched.sub_off[t_, c] <= soff
                                < sched.sub_off[t_, c] + sched.cap[t_, c]):
                            tl = tt
                            break
                    nc.tensor.matmul(
                        psums[tl],
                        lhsT=oh_t[:, j * P : (j + 1) * P],
                        rhs=mb_src[:, j, :],
                        start=(gblk == first[tl]),
                        stop=(gblk == last[tl]),
                    )

            def psum_packs(tpg):
                # row-major [128, tpg*d] f32 accumulator pack (<=1 bank)
                assert tpg * d * 4 <= 2048
                return pap.tile([P, tpg * d], f32, tag="acc", name="accpk")

            def spmm_group(sched, g, t_idx, t_oh, table_aps, val_t, psums,
                           bf_gather):
                if sched.group_blocks[g] == 0:
                    return
                grp = load_rl(sched, g, val_t)
                for c in range(sched.nch):
                    spmm_region(sched, g, c, grp, t_idx, t_oh, table_aps,
                                psums, bf_gather)

            # ================= L1 (with INFO interleaved) =================
            user_chunks = [
                t_user.ap()[c * cfg.ch_u : min((c + 1) * cfg.ch_u, cfg.n_user), :d]
                for c in range(cfg.nch_u)
            ]
            itemp_chunks = [
                t_itemp.ap()[c * cfg.ch_i : min((c + 1) * cfg.ch_i, cfg.n_item), :d]
                for c in range(cfg.nch_i)
            ]

            def info_group(gi):
                # INFO SpMM is independent of the AllGather chain; interleave
                # its groups into the L1 loop to fill gather-queue idle time.
                pack = ipap.tile([P, si.tpg * d], f32, tag="iacc")
                psums = [pack[:, tl * d : (tl + 1) * d] for tl in range(si.tpg)]
                spmm_group(si, gi, t_ini, t_ino, itemp_chunks,
                           t_vals.get("in"), psums, bf_gather=True)
                for tl in range(si.tpg):
                    T = gi * si.tpg + tl
                    nc.vector.tensor_copy(t3R[:, T, :], psums[tl])

            # L2 setup: regions interleave into the L1 loop as soon as the
            # AllGather chunks behind each h1 chunk land (chunk c needs AG
            # k<=ceil(((c+1)*ch_h)/(nc*cr))-1; schedule with 1-group margin).
            h1_chunks = [
                h1_full[c * cfg.ch_h : min((c + 1) * cfg.ch_h, cfg.flat_h1), :d]
                for c in range(cfg.nch_h)
            ]
            # two groups share one bank-sized [128, 512] f32 pack
            assert s2.ng % 2 == 0 and 2 * s2.tpg * d * 4 <= 2048
            l2_packs = [
                l2ap.tile([P, 2 * s2.tpg * d], f32, tag="l2acc", name="l2pk")
                for _ in range(s2.ng // 2)
            ]
            l2_psums = [
                [
                    l2_packs[g // 2][
                        :, ((g % 2) * s2.tpg + tl) * d
                        : ((g % 2) * s2.tpg + tl + 1) * d
                    ]
                    for tl in range(s2.tpg)
                ]
                for g in range(s2.ng)
            ]
            l2_grps = [
                load_rl(s2, g, t_vals.get("l2")) for g in range(s2.ng)
            ]

            def l2_phase(c):
                for g in range(s2.ng):
                    if s2.group_blocks[g] == 0:
                        continue
                    spmm_region(s2, g, c, l2_grps[g], t_l2i, t_l2o, h1_chunks,
                                l2_psums[g], bf_gather=True)

            info_after = {2: 0, 4: 1, 8: 2, 11: 3} if s1.ng >= 12 else {}
            l2_after = {}
            info_done = set()
            l2_done = set()
            for g in range(s1.ng):
                pack = psum_packs(s1.tpg)
                psums = [pack[:, tl * d : (tl + 1) * d] for tl in range(s1.tpg)]
                spmm_group(
                    s1, g, t_l1i, t_l1o, user_chunks,
                    t_vals.get("l1"), psums, bf_gather=True,
                )
                u0_t = u0p.tile([P, s1.tpg, d], f32, tag="u0t")
                r0 = g * s1.tpg * P
                nc.sync.dma_start(
                    u0_t[:],
                    t_u0s.ap()[r0 : r0 + s1.tpg * P, :].rearrange(
                        "(t p) d -> p t d", p=P
                    ),
                )
                for tl in range(s1.tpg):
                    gt = g * s1.tpg + tl
                    h1_t = hp.tile([P, d], f32, tag="hrow")
                    nc.vector.scalar_tensor_tensor(
                        out=h1_t[:],
                        in0=psums[tl],
                        scalar=ist_t[:, gt : gt + 1],
                        in1=u0_t[:, tl, :],
                        op0=mybir.AluOpType.mult,
                        op1=mybir.AluOpType.add,
                    )
                    k = g // cfg.gpa
                    lrow = ((g % cfg.gpa) * s1.tpg + tl) * P
                    h1_b16 = hp.tile([P, d], bf, tag="hrowb")
                    nc.scalar.copy(h1_b16[:], h1_t[:])
                    nc.sync.dma_start(
                        h1ag[k][lrow : lrow + P, :], h1_b16[:]
                    )
                    nc.sync.dma_start(
                        h1_shard[gt * P : (gt + 1) * P, :], h1_t[:]
                    )
                if (g + 1) % cfg.gpa == 0:
                    k = g // cfg.gpa
                    o0 = k * nc_ * cfg.cr
                    nc.gpsimd.collective_compute(
                        "AllGather",
                        mybir.AluOpType.bypass,
                        replica_groups=[list(range(nc_))],
                        ins=[h1ag[k][:].opt()],
                        outs=[h1fb[k][:].opt()],
                    )
                    # expand AG output into the 256B-stride padded bf16
                    # gather table, pipelined per AG chunk
                    nc.gpsimd.dma_start(
                        h1_full[o0 : o0 + nc_ * cfg.cr, :d], h1fb[k][:]
                    )
                if g in info_after:
                    info_group(info_after[g])
                    info_done.add(info_after[g])
                if g in l2_after:
                    l2_phase(l2_after[g])
                    l2_done.add(l2_after[g])

            # h1 batch rows gather (from own shard)
            h1bi_t = pp.tile([P, ubp // 16], mybir.dt.int16, tag="h1bidx")
            nc.sync.dma_start(h1bi_t[:], t_h1bi.ap())
            for s0 in range(0, ubp, GSUB):
                n = min(GSUB, ubp - s0)
                nc.gpsimd.dma_gather(
                    h1b_t[:, s0 // P : (s0 + n) // P, :],
                    h1_shard[:],
                    h1bi_t[:, s0 // 16 : (s0 + n) // 16],
                    n, n, d, single_packet=False,
                    queue_num=next_q(),
                )

            # ================= INFO (groups not yet interleaved) ============
            for gi in range(si.ng):
                if gi not in info_done:
                    info_group(gi)

            # ================= L2 (remaining chunk phases + drains) =========
            for c in range(s2.nch):
                if c not in l2_done:
                    l2_phase(c)
            for g in range(s2.ng):
                psums = l2_psums[g]
                for tl in range(s2.tpg):
                    T = g * s2.tpg + tl
                    x1 = hp.tile([P, d], f32, tag="hrow")
                    nc.vector.tensor_scalar_mul(
                        x1[:], psums[tl], isb_t[:, T : T + 1]
                    )
                    x2 = hp.tile([P, d], f32, tag="hrow")
                    nc.vector.scalar_tensor_tensor(
                        out=x2[:], in0=t3R[:, T, :],
                        scalar=iib_t[:, T : T + 1], in1=x1[:],
                        op0=mybir.AluOpType.mult, op1=mybir.AluOpType.add,
                    )
                    fu = hp.tile([P, d], f32, tag="hrow")
                    nc.vector.scalar_tensor_tensor(
                        out=fu[:], in0=h1b_t[:, T, :], scalar=2.0, in1=x2[:],
                        op0=mybir.AluOpType.mult, op1=mybir.AluOpType.add,
                    )
                    nc.sync.dma_start(fu_tab[T * P : (T + 1) * P, :], fu[:])

            # ================= FINAL =================
            item_chunks_f32 = [
                t_item.ap()[c * cfg.ch_i : min((c + 1) * cfg.ch_i, cfg.n_item), :]
                for c in range(cfg.nch_i)
            ]
            sc_t = pp.tile([P, ftot // P], f32, tag="scores")
            for chn in range(cfg.nch_i):
                n = int(fcap[chn])
                s0 = int(fbase[chn])
                fb = n // P
                iu = idxp.tile([P, n // 16], mybir.dt.int16, tag="idx")
                nc.sync.dma_start(
                    iu[:], t_pui.ap()[:, s0 // 16 : (s0 + n) // 16]
                )
                ii = idxp.tile([P, n // 16], mybir.dt.int16, tag="idx")
                nc.sync.dma_start(
                    ii[:], t_pii.ap()[:, s0 // 16 : (s0 + n) // 16]
                )
                u_t = mfp.tile([P, fb, d], f32, tag="msgf")
                v_t = mfp.tile([P, fb, d], f32, tag="msgf")
                for q0 in range(0, n, GSUB):
                    nq = min(GSUB, n - q0)
                    nc.gpsimd.dma_gather(
                        u_t[:, q0 // P : (q0 + nq) // P, :], fu_tab[:],
                        iu[:, q0 // 16 : (q0 + nq) // 16], nq, nq, d,
                        single_packet=False, queue_num=next_q(),
                    )
                    nc.gpsimd.dma_gather(
                        v_t[:, q0 // P : (q0 + nq) // P, :], item_chunks_f32[chn],
                        ii[:, q0 // 16 : (q0 + nq) // 16], nq, nq, d,
                        single_packet=False, queue_num=next_q(),
                    )
                pr = mfp.tile([P, fb, d], f32, tag="prod")
                nc.vector.tensor_mul(pr[:], u_t[:], v_t[:])
                dot = hp.tile([P, fb], f32, tag="dot")
                nc.vector.tensor_reduce(
                    dot[:], pr[:], axis=mybir.AxisListType.X,
                    op=mybir.AluOpType.add,
                )
                nc.scalar.activation(
                    sc_t[:, s0 // P : (s0 + n) // P], dot[:],
                    mybir.ActivationFunctionType.Sigmoid, scale=2.0,
                )
            nc.sync.dma_start(t_scores.ap(), sc_t[:])

    nc.compile()
    return nc


_CACHE = {}


def _run(cfg, inputs, trace=False):
    import time as _time

    _t = _time.time()
    plan, in_maps, out_meta = _prep(cfg, inputs)
    print(f"[kernel] prep: {_time.time()-_t:.1f}s", flush=True)
    _t = _time.time()
    key = (
        cfg.n_user, plan["s1"].total_slots, plan["s2"].total_slots,
        plan["si"].total_slots, plan["ubt_p"], plan["ftot"], plan["ones"],
    )
    if key not in _CACHE:
        _CACHE[key] = _build_program(plan)
        print(f"[kernel] build+compile: {_time.time()-_t:.1f}s", flush=True)
    nc = _CACHE[key]
    _t = _time.time()
    kw = {}
    if trace:
        # single-core NTFF (SPMD cores are balanced); exec_time_ns comes back
        kw = dict(trace=True, trace_cores=[0])
    res = run_bass_kernel_spmd(
        nc, in_maps, core_ids=list(range(cfg.nc)), **kw
    )
    print(f"[kernel] run: {_time.time()-_t:.1f}s", flush=True)
    out = np.zeros(len(inputs["user_ids"]), np.float32)
    for c in range(cfg.nc):
        js, slots = out_meta[c]
        sc = res.results[c]["scores"]
        out[js] = sc[slots % P, slots // P]
    return out, res


def kernel(**inputs):
    out, _ = _run(REAL, inputs, trace=bool(os.environ.get("KERNEL_TRACE")))
    return out


# revision 45
# speedup vs baseline: 1.0721x; 1.0721x over previous
"""DiffNet GNN message-passing kernel for 8 Trainium2 NeuronCores.

Math: final_user = t2/deg_soc + 2*h1 + t3/deg_info, restricted to batch users,
where h1 = A_soc@u0/deg_soc + u0 (needed for ALL users since layer 2 gathers
arbitrary columns), t2 = A_soc@h1 at batch rows only, t3 = A_info@item_emb at
batch rows only. Output = sigmoid(2 * sum(final_user[uids] * item_emb[iids])).

Sharding: by output row range (12500 users/core). Layer-1 SpMM over the full
edge set partitions exactly by row; one chunked AllGather publishes h1; layer-2
and info SpMMs run only on each core's batch-user rows.

Device SpMM: edges sorted by (group, col-chunk, tile, col); per-edge message
rows pulled from HBM by SWDGE dma_gather round-robined over 4 SWDGE queues
(each queue runs on its own GPSIMD DSP core pair, so descriptor generation for
4 gathers proceeds concurrently). user/item/h1 tables are bf16 padded to a
256B row stride so each gather descriptor moves only 128B. Segment-sum: the
one-hot routing matrices are precomputed on host in fp8 and streamed from HBM
(DVE is_equal runs at the errata-limited 1x rate and was a ~1ms bottleneck);
the fp8 one-hot is the PE stationary [128e,128r], bf16 messages move
[128e,64d], accumulating row-major [128r,64d] tiles in PSUM — no transposes.
The INFO SpMM is interleaved into the L1 loop to fill gather-queue idle time.
"""

import sys

sys.path.insert(0, "/opt/trn_rl_repo")

import math
import os

import numpy as np
import ml_dtypes

import concourse.bacc as bacc
import concourse.bass as bass
import concourse.mybir as mybir
import concourse.tile as tile
from concourse.bass_utils import run_bass_kernel_spmd

P = 128
BF16 = ml_dtypes.bfloat16
GSUB = 4096  # max idxs per dma_gather sub-call (split across SWDGE queues)


class Cfg:
    def __init__(self, n_user, n_item, d, n_cores, tpg1, gpa, tpg2, chunk):
        self.n_user = n_user
        self.n_item = n_item
        self.d = d
        self.nc = n_cores
        self.rpc = n_user // n_cores  # rows per core
        t1 = -(-self.rpc // P)  # L1 tiles per core (unpadded)
        self.tpg1 = tpg1  # L1 tiles per group
        self.t1p = -(-t1 // tpg1) * tpg1  # padded L1 tile count
        self.ng1 = self.t1p // tpg1
        self.gpa = gpa  # groups per AllGather chunk
        assert self.ng1 % gpa == 0
        self.agc = self.ng1 // gpa  # number of AG chunks
        self.cr = tpg1 * P * gpa  # rows per core per AG chunk
        self.shard_rows = self.t1p * P
        self.flat_h1 = self.nc * self.shard_rows  # h1_full rows
        self.tpg2 = tpg2  # batch tiles per group (L2 & info)
        self.chunk = chunk  # max gather-chunk rows (int16 limit)
        self.nch_u = -(-n_user // chunk)
        self.ch_u = -(-n_user // self.nch_u)
        self.nch_i = -(-n_item // chunk)
        self.ch_i = -(-n_item // self.nch_i)
        self.nch_h = -(-self.flat_h1 // chunk)
        self.ch_h = -(-self.flat_h1 // self.nch_h)


REAL = Cfg(100000, 50000, 64, 8, 7, 2, 4, 25088)


def _wrap_idx(idx_call):
    """[n] int16 -> [128, n/16] wrapped+replicated."""
    n = idx_call.shape[0]
    a = idx_call.reshape(n // 16, 16).T  # [16, n/16]
    return np.tile(a, (8, 1))


def _gather_raw(nc, out_ap, in_ap, idxs_ap, num_idxs, elem_size, elem_step,
                queue_num):
    """dma_gather without the 256B-payload restriction.

    in_ap rows live at a 256B stride (elem_step elements) but only elem_size
    elements (128B for bf16 d=64) are moved per descriptor.
    """
    g = nc.gpsimd
    stride_bytes = elem_step * mybir.dt.size(in_ap.dtype)
    assert stride_bytes % 256 == 0
    _in_ap = g.lower_ap_dma(in_ap, for_custom_bir_dma=True)
    _idxs_ap = g.lower_ap(idxs_ap)
    _out_ap = g.lower_ap(out_ap)
    return g.add_instruction(
        mybir.InstDMAGatherAnt(
            name=g.bass.get_next_instruction_name(),
            ins=[*_in_ap, _idxs_ap, g.lower_val_access(g.to_reg(num_idxs))],
            outs=[_out_ap],
            transpose=False,
            num_idxs=num_idxs,
            elem_size=elem_size,
            stride_bytes_256=stride_bytes // 256,
            gen_mode=0,
            single_packet=False,
            queue_num=queue_num,
            sbuf_tokens_per_rank=0,
            sbuf_free_dim_per_rank=0,
            sbuf_free_dim_pad_per_rank=0,
            sbuf_byte_offset=0,
        )
    )


class SpmmSched:
    """SPMD-uniform slot/block layout for one SpMM (same across cores)."""

    def __init__(self, ntp, tpg, nch):
        self.ntp = ntp  # padded tile count
        self.tpg = tpg
        self.ng = ntp // tpg
        self.nch = nch
        self.cap = None  # [ntp, nch] slots, multiples of 128

    def finalize(self):
        ntp, tpg, ng, nch = self.ntp, self.tpg, self.ng, self.nch
        cap = self.cap
        # ensure every tile has >=1 block so its PSUM region gets zeroed
        for t in range(ntp):
            if cap[t].sum() == 0:
                cap[t, 0] = P
        # region = (g, c): tiles g*tpg..g*tpg+tpg-1
        self.sub_off = np.zeros((ntp, nch), np.int64)  # slot offset in region
        self.region_nidx = np.zeros((ng, nch), np.int64)
        self.slot_base = np.zeros((ng, nch), np.int64)  # global slot offset
        self.blk_base = np.zeros((ng, nch), np.int64)
        self.group_blk0 = np.zeros(ng, np.int64)
        s = 0
        b = 0
        for g in range(ng):
            self.group_blk0[g] = b
            for c in range(nch):
                self.slot_base[g, c] = s
                self.blk_base[g, c] = b
                off = 0
                for tl in range(tpg):
                    t = g * tpg + tl
                    self.sub_off[t, c] = off
                    off += cap[t, c]
                self.region_nidx[g, c] = off
                s += off
                b += off // P
        self.total_slots = s
        self.total_blocks = b
        self.group_blocks = [
            int(sum(self.region_nidx[g]) // P) for g in range(ng)
        ]
        # per (g, tl): ordered list of global block ids (for start/stop flags)
        self.tile_blocks = {}
        for g in range(ng):
            for tl in range(self.tpg):
                t = g * self.tpg + tl
                blks = []
                for c in range(nch):
                    b0 = self.blk_base[g, c] + self.sub_off[t, c] // P
                    blks += list(range(b0, b0 + cap[t, c] // P))
                self.tile_blocks[(g, tl)] = blks
        # idx array column offsets (global, in units of 16 slots)
        self.idx_off = np.zeros((ng, nch), np.int64)
        w = 0
        for g in range(ng):
            for c in range(nch):
                self.idx_off[g, c] = w
                w += self.region_nidx[g, c] // 16
        self.idx_w = w


def _sched_caps(sched, per_core_tc_counts):
    """per_core_tc_counts: list of [ntp, nch] arrays -> set caps."""
    mx = np.maximum.reduce(per_core_tc_counts)
    sched.cap = (-(-mx // P) * P).astype(np.int64)
    sched.finalize()


def _fill_spmm(sched, rows_t, cols_c, col_idx, rowloc, vals):
    """Place one core's edges into the schedule's slot space.

    rows_t: tile id per edge; cols_c: chunk id; col_idx: int16 local col;
    rowloc: row-in-tile (0..127); vals: edge values (float32).
    Edges are sorted by column within each (tile, chunk) so the gather's HBM
    addresses ascend (DRAM locality).
    Returns (idx_arr [128, idx_w] i16, rl [128, B] bf16, val_w [128, B] f32,
             counts [ng*nch] i32).
    """
    ntp, tpg, ng, nch = sched.ntp, sched.tpg, sched.ng, sched.nch
    g_e = rows_t // tpg
    tl_e = rows_t % tpg
    bid = (g_e * nch + cols_c) * tpg + tl_e
    order = np.lexsort((col_idx, bid))
    bid_s = bid[order]
    counts = np.bincount(bid_s, minlength=ng * nch * tpg)
    starts = np.concatenate([[0], np.cumsum(counts)[:-1]])
    rank = np.arange(len(bid_s)) - starts[bid_s]
    t_s = rows_t[order]
    c_s = cols_c[order]
    g_s = g_e[order]
    slot = (
        sched.slot_base[g_s, c_s]
        + sched.sub_off[t_s, c_s]
        + rank
    )
    ns = sched.total_slots
    idx_flat = np.zeros(ns, np.int32)
    rl_flat = np.full(ns, -1.0, np.float32)
    val_flat = np.zeros(ns, np.float32)
    idx_flat[slot] = col_idx[order]
    rl_flat[slot] = rowloc[order]
    val_flat[slot] = vals[order]
    # pad slots keep idx 0 (real harmless gathers) so every slot is always
    # written -- avoids NaN garbage flowing into the matmul.
    call_counts = np.zeros(ng * nch, np.int32)
    # wrap
    idx_arr = np.empty((P, sched.idx_w), np.int16)
    for g in range(ng):
        for c in range(nch):
            n = sched.region_nidx[g, c]
            if n == 0:
                continue
            s0 = sched.slot_base[g, c]
            w0 = sched.idx_off[g, c]
            idx_arr[:, w0 : w0 + n // 16] = _wrap_idx(
                idx_flat[s0 : s0 + n].astype(np.int16)
            )
    # host-precomputed one-hot routing matrices, fp8 (exact for 0/1):
    # oh[e, b*128 + r] = 1 iff edge in slot (b, e) targets row r of its tile.
    # Streamed from HBM instead of generated on DVE (is_equal runs at the
    # errata-limited 1x rate and was a ~1ms bottleneck).
    FP8 = mybir.dt.np(mybir.dt.float8e4)
    rl_i = rl_flat.reshape(sched.total_blocks, P).T.astype(np.int32)  # [P, B]
    oh = (rl_i[:, :, None] == np.arange(P, dtype=np.int32)[None, None, :])
    oh = np.ascontiguousarray(oh.astype(FP8).reshape(P, sched.total_blocks * P))
    val_w = np.ascontiguousarray(val_flat.reshape(sched.total_blocks, P).T)
    return idx_arr, oh, val_w, call_counts


def _prep(cfg, inputs):
    """All host-side preprocessing. Returns (plan, in_maps, out_meta)."""
    nc_, d = cfg.nc, cfg.d
    user_emb = np.asarray(inputs["user_emb"], np.float32)
    item_emb = np.asarray(inputs["item_emb"], np.float32)
    s_rows = np.asarray(inputs["social_rows"], np.int64)
    s_cols = np.asarray(inputs["social_cols"], np.int64)
    s_vals = np.asarray(inputs["social_vals"], np.float32)
    i_rows = np.asarray(inputs["info_rows"], np.int64)
    i_cols = np.asarray(inputs["info_cols"], np.int64)
    i_vals = np.asarray(inputs["info_vals"], np.float32)
    uids = np.asarray(inputs["user_ids"], np.int64)
    iids = np.asarray(inputs["item_ids"], np.int64)
    eps = 1e-8

    ones = bool(np.all(s_vals == 1.0) and np.all(i_vals == 1.0))

    deg_soc = np.bincount(s_rows, weights=s_vals, minlength=cfg.n_user)
    deg_info = np.bincount(i_rows, weights=i_vals, minlength=cfg.n_user)
    inv_soc = (1.0 / (deg_soc.astype(np.float32) + eps)).astype(np.float32)
    inv_info = (1.0 / (deg_info.astype(np.float32) + eps)).astype(np.float32)

    # padded bf16 tables: 256B row stride, payload in cols [:64]
    user_pad = np.zeros((cfg.n_user, 2 * d), BF16)
    user_pad[:, :d] = user_emb
    item_pad = np.zeros((cfg.n_item, 2 * d), BF16)
    item_pad[:, :d] = item_emb

    # batch users
    uniq = np.unique(uids)
    owner = uniq // cfg.rpc
    bu = [uniq[owner == c] for c in range(nc_)]
    ubmax = max(len(b) for b in bu)
    ubt = -(-ubmax // P)
    ng2 = max(1, -(-ubt // cfg.tpg2))
    ubt_p = ng2 * cfg.tpg2
    ubp = ubt_p * P

    # --- L1 schedule ---
    s1 = SpmmSched(cfg.t1p, cfg.tpg1, cfg.nch_u)
    order = np.argsort(s_rows, kind="stable")
    sr, sc, sv = s_rows[order], s_cols[order], s_vals[order]
    bounds = np.searchsorted(sr, [c * cfg.rpc for c in range(nc_ + 1)])
    core_l1 = []
    tc_counts = []
    for c in range(nc_):
        lo, hi = bounds[c], bounds[c + 1]
        lr = sr[lo:hi] - c * cfg.rpc
        col = sc[lo:hi]
        t = lr // P
        ch = col // cfg.ch_u
        core_l1.append((t, ch, (col - ch * cfg.ch_u), lr % P, sv[lo:hi]))
        m = np.zeros((cfg.t1p, cfg.nch_u), np.int64)
        np.add.at(m, (t, ch), 1)
        tc_counts.append(m)
    _sched_caps(s1, tc_counts)

    # --- L2 & info: batch-row-restricted ---
    slot_of = np.full(cfg.n_user, -1, np.int64)
    for c in range(nc_):
        slot_of[bu[c]] = np.arange(len(bu[c]))
    in_batch = slot_of >= 0

    def batch_edges(rows, cols, vals):
        m = in_batch[rows]
        r, co, v = rows[m], cols[m], vals[m]
        core = r // cfg.rpc
        return r, co, v, core

    s2 = SpmmSched(ubt_p, cfg.tpg2, cfg.nch_h)
    si = SpmmSched(ubt_p, cfg.tpg2, cfg.nch_i)

    def h1_flat(col):
        own = col // cfg.rpc
        lr = col - own * cfg.rpc
        k = lr // cfg.cr
        off = lr - k * cfg.cr
        return k * (nc_ * cfg.cr) + own * cfg.cr + off

    r2, c2, v2, core2 = batch_edges(sr, sc, sv)
    f2 = h1_flat(c2)
    ri, ci, vi, corei = batch_edges(i_rows, i_cols, i_vals)

    core_l2, core_in = [], []
    tc2, tci = [], []
    for c in range(nc_):
        m = core2 == c
        sl = slot_of[r2[m]]
        t = sl // P
        ch = f2[m] // cfg.ch_h
        core_l2.append((t, ch, f2[m] - ch * cfg.ch_h, sl % P, v2[m]))
        a = np.zeros((ubt_p, cfg.nch_h), np.int64)
        np.add.at(a, (t, ch), 1)
        tc2.append(a)
        m = corei == c
        sl = slot_of[ri[m]]
        t = sl // P
        ch = ci[m] // cfg.ch_i
        core_in.append((t, ch, ci[m] - ch * cfg.ch_i, sl % P, vi[m]))
        a = np.zeros((ubt_p, cfg.nch_i), np.int64)
        np.add.at(a, (t, ch), 1)
        tci.append(a)
    _sched_caps(s2, tc2)
    _sched_caps(si, tci)

    # --- final pairs ---
    pcore = uids // cfg.rpc
    pch = iids // cfg.ch_i
    fcap = np.zeros(cfg.nch_i, np.int64)
    per_core_pairs = []
    for c in range(nc_):
        m = np.nonzero(pcore == c)[0]
        o = m[np.lexsort((iids[m], pch[m]))]
        per_core_pairs.append(o)
        cnts = np.bincount(pch[o], minlength=cfg.nch_i)
        fcap = np.maximum(fcap, cnts)
    fcap = -(-fcap // P) * P
    fcap = np.maximum(fcap, P)
    fbase = np.concatenate([[0], np.cumsum(fcap)])
    ftot = int(fbase[-1])

    plan = dict(
        cfg=cfg, s1=s1, s2=s2, si=si, ubt_p=ubt_p, ubp=ubp, ng2=ng2,
        fcap=fcap, fbase=fbase, ftot=ftot, ones=ones,
    )

    in_maps = []
    out_meta = []  # per core: (pair_js, slots)
    for c in range(nc_):
        t, ch, cidx, rl, v = core_l1[c]
        l1_idx, l1_oh, l1_val, cnt1 = _fill_spmm(s1, t, ch, cidx, rl, v)
        t, ch, cidx, rl, v = core_l2[c]
        l2_idx, l2_oh, l2_val, cnt2 = _fill_spmm(s2, t, ch, cidx, rl, v)
        t, ch, cidx, rl, v = core_in[c]
        in_idx, in_oh, in_val, cnti = _fill_spmm(si, t, ch, cidx, rl, v)

        # u0 shard
        u0s = np.zeros((cfg.shard_rows, d), np.float32)
        nrow = min(cfg.rpc, cfg.n_user - c * cfg.rpc)
        u0s[:nrow] = user_emb[c * cfg.rpc : c * cfg.rpc + nrow]

        # invdeg arrays
        ist = np.zeros((P, cfg.t1p), np.float32)
        rows = c * cfg.rpc + np.arange(nrow)
        ist[np.arange(nrow) % P, np.arange(nrow) // P] = inv_soc[rows]
        isb = np.zeros((P, ubt_p), np.float32)
        iib = np.zeros((P, ubt_p), np.float32)
        nb = len(bu[c])
        isb[np.arange(nb) % P, np.arange(nb) // P] = inv_soc[bu[c]]
        iib[np.arange(nb) % P, np.arange(nb) // P] = inv_info[bu[c]]

        # h1 batch gather idx (local shard rows); pads gather row 0
        h1b = np.zeros(ubp, np.int16)
        h1b[:nb] = (bu[c] - c * cfg.rpc).astype(np.int16)

        # final pairs
        o = per_core_pairs[c]
        pu = np.zeros(ftot, np.int16)
        pi = np.zeros(ftot, np.int16)
        slots = np.empty(len(o), np.int64)
        pos = 0
        for chn in range(cfg.nch_i):
            sel = o[pch[o] == chn]
            k = len(sel)
            s0 = fbase[chn]
            pu[s0 : s0 + k] = slot_of[uids[sel]].astype(np.int16)
            pi[s0 : s0 + k] = (iids[sel] - chn * cfg.ch_i).astype(np.int16)
            slots[pos : pos + k] = s0 + np.arange(k)
            pos += k
        out_meta.append((o, slots))

        m = {
            "user_pad": user_pad,
            "item_pad": item_pad,
            "item_emb": item_emb,
            "u0s": u0s,
            "l1_idx": l1_idx, "l1_oh": l1_oh,
            "l2_idx": l2_idx, "l2_oh": l2_oh,
            "in_idx": in_idx, "in_oh": in_oh,
            "ist": ist, "isb": isb, "iib": iib,
            "h1b_idx": _wrap_idx(h1b),
            "pu_idx": _wrap_idx(pu), "pi_idx": _wrap_idx(pi),
        }
        if not ones:
            m["l1_val"] = l1_val
            m["l2_val"] = l2_val
            m["in_val"] = in_val
        in_maps.append(m)
    return plan, in_maps, out_meta


def _build_program(plan):
    cfg = plan["cfg"]
    s1, s2, si = plan["s1"], plan["s2"], plan["si"]
    ubt_p, ubp, ng2 = plan["ubt_p"], plan["ubp"], plan["ng2"]
    fcap, fbase, ftot = plan["fcap"], plan["fbase"], plan["ftot"]
    ones = plan["ones"]
    d = cfg.d
    nc_ = cfg.nc
    f32 = mybir.dt.float32
    bf = mybir.dt.bfloat16

    # 4 SWDGE queues: each runs on its own GPSIMD DSP core pair, so gathers
    # on different queues generate descriptors concurrently.
    nc = bacc.Bacc("TRN2", debug=False, num_devices=nc_, num_swdge_queues=4)
    qrr = [0]

    def next_q():
        qrr[0] = (qrr[0] + 1) % 4
        return qrr[0]

    t_user = nc.dram_tensor("user_pad", [cfg.n_user, 2 * d], bf, kind="ExternalInput")
    t_itemp = nc.dram_tensor("item_pad", [cfg.n_item, 2 * d], bf, kind="ExternalInput")
    t_item = nc.dram_tensor("item_emb", [cfg.n_item, d], f32, kind="ExternalInput")
    t_u0s = nc.dram_tensor("u0s", [cfg.shard_rows, d], f32, kind="ExternalInput")
    fp8 = mybir.dt.float8e4
    t_l1i = nc.dram_tensor("l1_idx", [P, s1.idx_w], mybir.dt.int16, kind="ExternalInput")
    t_l1o = nc.dram_tensor("l1_oh", [P, s1.total_blocks * P], fp8, kind="ExternalInput")
    t_l2i = nc.dram_tensor("l2_idx", [P, s2.idx_w], mybir.dt.int16, kind="ExternalInput")
    t_l2o = nc.dram_tensor("l2_oh", [P, s2.total_blocks * P], fp8, kind="ExternalInput")
    t_ini = nc.dram_tensor("in_idx", [P, si.idx_w], mybir.dt.int16, kind="ExternalInput")
    t_ino = nc.dram_tensor("in_oh", [P, si.total_blocks * P], fp8, kind="ExternalInput")
    t_ist = nc.dram_tensor("ist", [P, cfg.t1p], f32, kind="ExternalInput")
    t_isb = nc.dram_tensor("isb", [P, ubt_p], f32, kind="ExternalInput")
    t_iib = nc.dram_tensor("iib", [P, ubt_p], f32, kind="ExternalInput")
    t_h1bi = nc.dram_tensor("h1b_idx", [P, ubp // 16], mybir.dt.int16, kind="ExternalInput")
    t_pui = nc.dram_tensor("pu_idx", [P, ftot // 16], mybir.dt.int16, kind="ExternalInput")
    t_pii = nc.dram_tensor("pi_idx", [P, ftot // 16], mybir.dt.int16, kind="ExternalInput")
    t_scores = nc.dram_tensor("scores", [P, ftot // P], f32, kind="ExternalOutput")
    t_vals = {}
    if not ones:
        t_vals["l1"] = nc.dram_tensor("l1_val", [P, s1.total_blocks], f32, kind="ExternalInput")
        t_vals["l2"] = nc.dram_tensor("l2_val", [P, s2.total_blocks], f32, kind="ExternalInput")
        t_vals["in"] = nc.dram_tensor("in_val", [P, si.total_blocks], f32, kind="ExternalInput")

    with tile.TileContext(nc) as tc:
        with (
            tc.tile_pool(name="const", bufs=1) as cp,
            tc.tile_pool(name="persist", bufs=1) as pp,
            tc.tile_pool(name="idx", bufs=10) as idxp,
            tc.tile_pool(name="msgs", bufs=8) as msgp,
            tc.tile_pool(name="msgf", bufs=2) as mfp,
            tc.tile_pool(name="oh", bufs=6) as ohp,
            tc.tile_pool(name="rl", bufs=6) as rlp,
            tc.tile_pool(name="u0t", bufs=2) as u0p,
            tc.tile_pool(name="hrow", bufs=4) as hp,
            tc.tile_pool(name="psacc", bufs=4, space="PSUM") as pap,
            tc.tile_pool(name="psinfo", bufs=2, space="PSUM") as ipap,
            tc.tile_pool(name="psl2", bufs=2, space="PSUM") as l2ap,
            tc.tile_pool(name="dram", bufs=1, space="DRAM") as dram,
        ):
            # ---- constants / persistent ----
            ist_t = pp.tile([P, cfg.t1p], f32, tag="ist")
            nc.sync.dma_start(ist_t[:], t_ist.ap())
            isb_t = pp.tile([P, ubt_p], f32, tag="isb")
            nc.sync.dma_start(isb_t[:], t_isb.ap())
            iib_t = pp.tile([P, ubt_p], f32, tag="iib")
            nc.sync.dma_start(iib_t[:], t_iib.ap())
            t3R = pp.tile([P, ubt_p, d], f32, tag="t3R")
            h1b_t = pp.tile([P, ubt_p, d], f32, tag="h1b")
            nc.vector.memzero(h1b_t[:])

            # internal DRAM
            h1ag = [
                dram.tile([cfg.cr, d], bf, tag=f"h1ag{k}", name=f"h1ag{k}")
                for k in range(cfg.agc)
            ]
            h1fb = [
                dram.tile([nc_ * cfg.cr, d], bf, tag=f"h1fb{k}",
                          name=f"h1fb{k}")
                for k in range(cfg.agc)
            ]
            h1_full = dram.tile([cfg.flat_h1, 2 * d], bf, tag="h1full")
            h1_shard = dram.tile([cfg.shard_rows, d], f32, tag="h1shard")
            fu_tab = dram.tile([ubp, d], f32, tag="futab")

            def load_rl(sched, g, val_t):
                gb0 = int(sched.group_blk0[g])
                gblocks = sched.group_blocks[g]
                vw_t = None
                if val_t is not None:
                    vw_t = rlp.tile([P, gblocks], f32, tag="vw")
                    nc.sync.dma_start(vw_t[:], val_t.ap()[:, gb0 : gb0 + gblocks])
                first = {tl: sched.tile_blocks[(g, tl)][0] for tl in range(sched.tpg)
                         if sched.tile_blocks[(g, tl)]}
                last = {tl: sched.tile_blocks[(g, tl)][-1] for tl in range(sched.tpg)
                        if sched.tile_blocks[(g, tl)]}
                return vw_t, gb0, first, last

            def spmm_region(sched, g, c, grp, t_idx, t_oh, table_aps, psums,
                            bf_gather):
                """Emit gathers/onehot-load/matmuls for one (group, chunk)
                region.

                psums[tl]: PSUM [128, d] accumulator slice for each tile.
                bf_gather: gather 128B bf16 payloads from a padded table
                (table_aps are [:, :64] views of 256B-stride bf16 tables);
                else classic 256B f32 dma_gather.
                """
                vw_t, gb0, first, last = grp
                nidx = int(sched.region_nidx[g, c])
                if nidx == 0:
                    return
                rb = nidx // P
                w0 = int(sched.idx_off[g, c])
                it = idxp.tile([P, nidx // 16], mybir.dt.int16, tag="idx")
                nc.sync.dma_start(it[:], t_idx.ap()[:, w0 : w0 + nidx // 16])
                # prefetch the fp8 one-hot stream so it overlaps the gathers
                b0 = int(sched.blk_base[g, c])
                oh_t = ohp.tile([P, rb * P], mybir.dt.float8e4, tag="oh")
                nc.sync.dma_start(
                    oh_t[:], t_oh.ap()[:, b0 * P : (b0 + rb) * P]
                )
                if bf_gather:
                    m_t = msgp.tile([P, rb, d], bf, tag="msgs")
                    for s0 in range(0, nidx, GSUB):
                        n = min(GSUB, nidx - s0)
                        _gather_raw(
                            nc,
                            m_t[:, s0 // P : (s0 + n) // P, :],
                            table_aps[c],
                            it[:, s0 // 16 : (s0 + n) // 16],
                            n, d, 2 * d, next_q(),
                        )
                else:
                    m_t = mfp.tile([P, rb, d], f32, tag="msgf")
                    for s0 in range(0, nidx, GSUB):
                        n = min(GSUB, nidx - s0)
                        nc.gpsimd.dma_gather(
                            m_t[:, s0 // P : (s0 + n) // P, :],
                            table_aps[c],
                            it[:, s0 // 16 : (s0 + n) // 16],
                            n, n, d, single_packet=False,
                            queue_num=next_q(),
                        )
                if vw_t is not None:
                    rboff0 = int(sched.blk_base[g, c]) - gb0
                    nc.vector.tensor_tensor(
                        out=m_t[:],
                        in0=m_t[:],
                        in1=vw_t[:, rboff0 : rboff0 + rb]
                        .unsqueeze(2)
                        .to_broadcast([P, rb, d]),
                        op=mybir.AluOpType.mult,
                    )
                mb_src = m_t
                if not bf_gather:
                    mb_t = msgp.tile([P, rb, d], bf, tag="msgs")
                    nc.scalar.copy(mb_t[:], m_t[:])
                    mb_src = mb_t
                for j in range(rb):
                    gblk = b0 + j
                    soff = j * P
                    tl = 0
                    for tt in range(sched.tpg):
                        t_ = g * sched.tpg + tt
                        if (sched.sub_off[t_, c] <= soff
                                < sched.sub_off[t_, c] + sched.cap[t_, c]):
                            tl = tt
                            break
                    nc.tensor.matmul(
                        psums[tl],
                        lhsT=oh_t[:, j * P : (j + 1) * P],
                        rhs=mb_src[:, j, :],
                        start=(gblk == first[tl]),
                        stop=(gblk == last[tl]),
                    )

            def psum_packs(tpg):
                # row-major [128, tpg*d] f32 accumulator pack (<=1 bank)
                assert tpg * d * 4 <= 2048
                return pap.tile([P, tpg * d], f32, tag="acc", name="accpk")

            def spmm_group(sched, g, t_idx, t_oh, table_aps, val_t, psums,
                           bf_gather):
                if sched.group_blocks[g] == 0:
                    return
                grp = load_rl(sched, g, val_t)
                for c in range(sched.nch):
                    spmm_region(sched, g, c, grp, t_idx, t_oh, table_aps,
                                psums, bf_gather)

            # ================= L1 (with INFO interleaved) =================
            user_chunks = [
                t_user.ap()[c * cfg.ch_u : min((c + 1) * cfg.ch_u, cfg.n_user), :d]
                for c in range(cfg.nch_u)
            ]
            itemp_chunks = [
                t_itemp.ap()[c * cfg.ch_i : min((c + 1) * cfg.ch_i, cfg.n_item), :d]
                for c in range(cfg.nch_i)
            ]

            def info_group(gi):
                # INFO SpMM is independent of the AllGather chain; interleave
                # its groups into the L1 loop to fill gather-queue idle time.
                pack = ipap.tile([P, si.tpg * d], f32, tag="iacc")
                psums = [pack[:, tl * d : (tl + 1) * d] for tl in range(si.tpg)]
                spmm_group(si, gi, t_ini, t_ino, itemp_chunks,
                           t_vals.get("in"), psums, bf_gather=True)
                for tl in range(si.tpg):
                    T = gi * si.tpg + tl
                    nc.vector.tensor_copy(t3R[:, T, :], psums[tl])

            # L2 setup: regions interleave into the L1 loop as soon as the
            # AllGather chunks behind each h1 chunk land (chunk c needs AG
            # k<=ceil(((c+1)*ch_h)/(nc*cr))-1; schedule with 1-group margin).
            h1_chunks = [
                h1_full[c * cfg.ch_h : min((c + 1) * cfg.ch_h, cfg.flat_h1), :d]
                for c in range(cfg.nch_h)
            ]
            # two groups share one bank-sized [128, 512] f32 pack
            assert s2.ng % 2 == 0 and 2 * s2.tpg * d * 4 <= 2048
            l2_packs = [
                l2ap.tile([P, 2 * s2.tpg * d], f32, tag="l2acc", name="l2pk")
                for _ in range(s2.ng // 2)
            ]
            l2_psums = [
                [
                    l2_packs[g // 2][
                        :, ((g % 2) * s2.tpg + tl) * d
                        : ((g % 2) * s2.tpg + tl + 1) * d
                    ]
                    for tl in range(s2.tpg)
                ]
                for g in range(s2.ng)
            ]
            l2_grps = [
                load_rl(s2, g, t_vals.get("l2")) for g in range(s2.ng)
            ]

            def l2_phase(c):
                for g in range(s2.ng):
                    if s2.group_blocks[g] == 0:
                        continue
                    spmm_region(s2, g, c, l2_grps[g], t_l2i, t_l2o, h1_chunks,
                                l2_psums[g], bf_gather=True)

            info_after = {2: 0, 4: 1, 8: 2, 11: 3} if s1.ng >= 12 else {}
            l2_after = {}
            info_done = set()
            l2_done = set()
            def fire_ag(k):
                # The collective waits on the h1ag drain writes; it is emitted
                # AFTER the next group's gathers are dispatched so that wait
                # overlaps gather execution instead of head-of-line blocking
                # the gpsimd queue.
                o0 = k * nc_ * cfg.cr
                nc.gpsimd.collective_compute(
                    "AllGather",
                    mybir.AluOpType.bypass,
                    replica_groups=[list(range(nc_))],
                    ins=[h1ag[k][:].opt()],
                    outs=[h1fb[k][:].opt()],
                )
                # expand AG output into the 256B-stride padded bf16
                # gather table, pipelined per AG chunk (sync engine: keeps
                # the gpsimd dispatch path clear)
                nc.sync.dma_start(
                    h1_full[o0 : o0 + nc_ * cfg.cr, :d], h1fb[k][:]
                )

            ag_pending = []
            for g in range(s1.ng):
                pack = psum_packs(s1.tpg)
                psums = [pack[:, tl * d : (tl + 1) * d] for tl in range(s1.tpg)]
                spmm_group(
                    s1, g, t_l1i, t_l1o, user_chunks,
                    t_vals.get("l1"), psums, bf_gather=True,
                )
                while ag_pending:
                    fire_ag(ag_pending.pop(0))
                u0_t = u0p.tile([P, s1.tpg, d], f32, tag="u0t")
                r0 = g * s1.tpg * P
                nc.sync.dma_start(
                    u0_t[:],
                    t_u0s.ap()[r0 : r0 + s1.tpg * P, :].rearrange(
                        "(t p) d -> p t d", p=P
                    ),
                )
                for tl in range(s1.tpg):
                    gt = g * s1.tpg + tl
                    h1_t = hp.tile([P, d], f32, tag="hrow")
                    nc.vector.scalar_tensor_tensor(
                        out=h1_t[:],
                        in0=psums[tl],
                        scalar=ist_t[:, gt : gt + 1],
                        in1=u0_t[:, tl, :],
                        op0=mybir.AluOpType.mult,
                        op1=mybir.AluOpType.add,
                    )
                    k = g // cfg.gpa
                    lrow = ((g % cfg.gpa) * s1.tpg + tl) * P
                    h1_b16 = hp.tile([P, d], bf, tag="hrowb")
                    nc.scalar.copy(h1_b16[:], h1_t[:])
                    nc.sync.dma_start(
                        h1ag[k][lrow : lrow + P, :], h1_b16[:]
                    )
                    nc.sync.dma_start(
                        h1_shard[gt * P : (gt + 1) * P, :], h1_t[:]
                    )
                if (g + 1) % cfg.gpa == 0:
                    ag_pending.append(g // cfg.gpa)
                if g in info_after:
                    info_group(info_after[g])
                    info_done.add(info_after[g])
                if g in l2_after:
                    l2_phase(l2_after[g])
                    l2_done.add(l2_after[g])

            while ag_pending:
                fire_ag(ag_pending.pop(0))

            # ================= INFO (groups not yet interleaved) ============
            for gi in range(si.ng):
                if gi not in info_done:
                    info_group(gi)

            # ================= L2 (remaining chunk phases + drains) =========
            for c in range(s2.nch):
                if c not in l2_done:
                    l2_phase(c)

            # h1 batch rows gather (from own shard): waits on the last L1
            # drain, so it is emitted after all other gathers are queued
            # (it only feeds the L2 drains below).
            h1bi_t = pp.tile([P, ubp // 16], mybir.dt.int16, tag="h1bidx")
            nc.sync.dma_start(h1bi_t[:], t_h1bi.ap())
            for s0 in range(0, ubp, GSUB):
                n = min(GSUB, ubp - s0)
                nc.gpsimd.dma_gather(
                    h1b_t[:, s0 // P : (s0 + n) // P, :],
                    h1_shard[:],
                    h1bi_t[:, s0 // 16 : (s0 + n) // 16],
                    n, n, d, single_packet=False,
                    queue_num=next_q(),
                )
            for g in range(s2.ng):
                psums = l2_psums[g]
                for tl in range(s2.tpg):
                    T = g * s2.tpg + tl
                    x1 = hp.tile([P, d], f32, tag="hrow")
                    nc.vector.tensor_scalar_mul(
                        x1[:], psums[tl], isb_t[:, T : T + 1]
                    )
                    x2 = hp.tile([P, d], f32, tag="hrow")
                    nc.vector.scalar_tensor_tensor(
                        out=x2[:], in0=t3R[:, T, :],
                        scalar=iib_t[:, T : T + 1], in1=x1[:],
                        op0=mybir.AluOpType.mult, op1=mybir.AluOpType.add,
                    )
                    fu = hp.tile([P, d], f32, tag="hrow")
                    nc.vector.scalar_tensor_tensor(
                        out=fu[:], in0=h1b_t[:, T, :], scalar=2.0, in1=x2[:],
                        op0=mybir.AluOpType.mult, op1=mybir.AluOpType.add,
                    )
                    nc.sync.dma_start(fu_tab[T * P : (T + 1) * P, :], fu[:])

            # ================= FINAL =================
            item_chunks_f32 = [
                t_item.ap()[c * cfg.ch_i : min((c + 1) * cfg.ch_i, cfg.n_item), :]
                for c in range(cfg.nch_i)
            ]
            sc_t = pp.tile([P, ftot // P], f32, tag="scores")
            for chn in range(cfg.nch_i):
                n = int(fcap[chn])
                s0 = int(fbase[chn])
                fb = n // P
                iu = idxp.tile([P, n // 16], mybir.dt.int16, tag="idx")
                nc.sync.dma_start(
                    iu[:], t_pui.ap()[:, s0 // 16 : (s0 + n) // 16]
                )
                ii = idxp.tile([P, n // 16], mybir.dt.int16, tag="idx")
                nc.sync.dma_start(
                    ii[:], t_pii.ap()[:, s0 // 16 : (s0 + n) // 16]
                )
                u_t = mfp.tile([P, fb, d], f32, tag="msgf")
                v_t = mfp.tile([P, fb, d], f32, tag="msgf")
                for q0 in range(0, n, GSUB):
                    nq = min(GSUB, n - q0)
                    nc.gpsimd.dma_gather(
                        u_t[:, q0 // P : (q0 + nq) // P, :], fu_tab[:],
                        iu[:, q0 // 16 : (q0 + nq) // 16], nq, nq, d,
                        single_packet=False, queue_num=next_q(),
                    )
                    nc.gpsimd.dma_gather(
                        v_t[:, q0 // P : (q0 + nq) // P, :], item_chunks_f32[chn],
                        ii[:, q0 // 16 : (q0 + nq) // 16], nq, nq, d,
                        single_packet=False, queue_num=next_q(),
                    )
                pr = mfp.tile([P, fb, d], f32, tag="prod")
                nc.vector.tensor_mul(pr[:], u_t[:], v_t[:])
                dot = hp.tile([P, fb], f32, tag="dot")
                nc.vector.tensor_reduce(
                    dot[:], pr[:], axis=mybir.AxisListType.X,
                    op=mybir.AluOpType.add,
                )
                nc.scalar.activation(
                    sc_t[:, s0 // P : (s0 + n) // P], dot[:],
                    mybir.ActivationFunctionType.Sigmoid, scale=2.0,
                )
            nc.sync.dma_start(t_scores.ap(), sc_t[:])

    nc.compile()
    return nc


_CACHE = {}


def _run(cfg, inputs, trace=False):
    import time as _time

    _t = _time.time()
    plan, in_maps, out_meta = _prep(cfg, inputs)
    print(f"[kernel] prep: {_time.time()-_t:.1f}s", flush=True)
    _t = _time.time()
    key = (
        cfg.n_user, plan["s1"].total_slots, plan["s2"].total_slots,
        plan["si"].total_slots, plan["ubt_p"], plan["ftot"], plan["ones"],
    )
    if key not in _CACHE:
        _CACHE[key] = _build_program(plan)
        print(f"[kernel] build+compile: {_time.time()-_t:.1f}s", flush=True)
    nc = _CACHE[key]
    _t = _time.time()
    kw = {}
    if trace:
        # single-core NTFF (SPMD cores are balanced); exec_time_ns comes back
        kw = dict(trace=True, trace_cores=[0])
    res = run_bass_kernel_spmd(
        nc, in_maps, core_ids=list(range(cfg.nc)), **kw
    )
    print(f"[kernel] run: {_time.time()-_t:.1f}s", flush=True)
    out = np.zeros(len(inputs["user_ids"]), np.float32)
    for c in range(cfg.nc):
        js, slots = out_meta[c]
        sc = res.results[c]["scores"]
        out[js] = sc[slots % P, slots // P]
    return out, res


def kernel(**inputs):
    out, _ = _run(REAL, inputs, trace=bool(os.environ.get("KERNEL_TRACE")))
    return out


# revision 48
# speedup vs baseline: 1.0745x; 1.0022x over previous
"""DiffNet GNN message-passing kernel for 8 Trainium2 NeuronCores.

Math: final_user = t2/deg_soc + 2*h1 + t3/deg_info, restricted to batch users,
where h1 = A_soc@u0/deg_soc + u0 (needed for ALL users since layer 2 gathers
arbitrary columns), t2 = A_soc@h1 at batch rows only, t3 = A_info@item_emb at
batch rows only. Output = sigmoid(2 * sum(final_user[uids] * item_emb[iids])).

Sharding: by output row range (12500 users/core). Layer-1 SpMM over the full
edge set partitions exactly by row; one chunked AllGather publishes h1; layer-2
and info SpMMs run only on each core's batch-user rows.

Device SpMM: edges sorted by (group, col-chunk, tile, col); per-edge message
rows pulled from HBM by SWDGE dma_gather round-robined over 4 SWDGE queues
(each queue runs on its own GPSIMD DSP core pair, so descriptor generation for
4 gathers proceeds concurrently). user/item/h1 tables are bf16 padded to a
256B row stride so each gather descriptor moves only 128B. Segment-sum: the
one-hot routing matrices are precomputed on host in fp8 and streamed from HBM
(DVE is_equal runs at the errata-limited 1x rate and was a ~1ms bottleneck);
the fp8 one-hot is the PE stationary [128e,128r], bf16 messages move
[128e,64d], accumulating row-major [128r,64d] tiles in PSUM — no transposes.
The INFO SpMM is interleaved into the L1 loop to fill gather-queue idle time.
"""

import sys

sys.path.insert(0, "/opt/trn_rl_repo")

import math
import os

import numpy as np
import ml_dtypes

import concourse.bacc as bacc
import concourse.bass as bass
import concourse.mybir as mybir
import concourse.tile as tile
from concourse.bass_utils import run_bass_kernel_spmd

P = 128
BF16 = ml_dtypes.bfloat16
GSUB = 2048  # max idxs per dma_gather sub-call (split across SWDGE queues)


class Cfg:
    def __init__(self, n_user, n_item, d, n_cores, tpg1, gpa, tpg2, chunk):
        self.n_user = n_user
        self.n_item = n_item
        self.d = d
        self.nc = n_cores
        self.rpc = n_user // n_cores  # rows per core
        t1 = -(-self.rpc // P)  # L1 tiles per core (unpadded)
        self.tpg1 = tpg1  # L1 tiles per group
        self.t1p = -(-t1 // tpg1) * tpg1  # padded L1 tile count
        self.ng1 = self.t1p // tpg1
        self.gpa = gpa  # groups per AllGather chunk
        assert self.ng1 % gpa == 0
        self.agc = self.ng1 // gpa  # number of AG chunks
        self.cr = tpg1 * P * gpa  # rows per core per AG chunk
        self.shard_rows = self.t1p * P
        self.flat_h1 = self.nc * self.shard_rows  # h1_full rows
        self.tpg2 = tpg2  # batch tiles per group (L2 & info)
        self.chunk = chunk  # max gather-chunk rows (int16 limit)
        self.nch_u = -(-n_user // chunk)
        self.ch_u = -(-n_user // self.nch_u)
        self.nch_i = -(-n_item // chunk)
        self.ch_i = -(-n_item // self.nch_i)
        self.nch_h = -(-self.flat_h1 // chunk)
        self.ch_h = -(-self.flat_h1 // self.nch_h)


REAL = Cfg(100000, 50000, 64, 8, 7, 2, 4, 25088)


def _wrap_idx(idx_call):
    """[n] int16 -> [128, n/16] wrapped+replicated."""
    n = idx_call.shape[0]
    a = idx_call.reshape(n // 16, 16).T  # [16, n/16]
    return np.tile(a, (8, 1))


def _gather_raw(nc, out_ap, in_ap, idxs_ap, num_idxs, elem_size, elem_step,
                queue_num):
    """dma_gather without the 256B-payload restriction.

    in_ap rows live at a 256B stride (elem_step elements) but only elem_size
    elements (128B for bf16 d=64) are moved per descriptor.
    """
    g = nc.gpsimd
    stride_bytes = elem_step * mybir.dt.size(in_ap.dtype)
    assert stride_bytes % 256 == 0
    _in_ap = g.lower_ap_dma(in_ap, for_custom_bir_dma=True)
    _idxs_ap = g.lower_ap(idxs_ap)
    _out_ap = g.lower_ap(out_ap)
    return g.add_instruction(
        mybir.InstDMAGatherAnt(
            name=g.bass.get_next_instruction_name(),
            ins=[*_in_ap, _idxs_ap, g.lower_val_access(g.to_reg(num_idxs))],
            outs=[_out_ap],
            transpose=False,
            num_idxs=num_idxs,
            elem_size=elem_size,
            stride_bytes_256=stride_bytes // 256,
            gen_mode=0,
            single_packet=False,
            queue_num=queue_num,
            sbuf_tokens_per_rank=0,
            sbuf_free_dim_per_rank=0,
            sbuf_free_dim_pad_per_rank=0,
            sbuf_byte_offset=0,
        )
    )


class SpmmSched:
    """SPMD-uniform slot/block layout for one SpMM (same across cores)."""

    def __init__(self, ntp, tpg, nch):
        self.ntp = ntp  # padded tile count
        self.tpg = tpg
        self.ng = ntp // tpg
        self.nch = nch
        self.cap = None  # [ntp, nch] slots, multiples of 128

    def finalize(self):
        ntp, tpg, ng, nch = self.ntp, self.tpg, self.ng, self.nch
        cap = self.cap
        # ensure every tile has >=1 block so its PSUM region gets zeroed
        for t in range(ntp):
            if cap[t].sum() == 0:
                cap[t, 0] = P
        # region = (g, c): tiles g*tpg..g*tpg+tpg-1
        self.sub_off = np.zeros((ntp, nch), np.int64)  # slot offset in region
        self.region_nidx = np.zeros((ng, nch), np.int64)
        self.slot_base = np.zeros((ng, nch), np.int64)  # global slot offset
        self.blk_base = np.zeros((ng, nch), np.int64)
        self.group_blk0 = np.zeros(ng, np.int64)
        s = 0
        b = 0
        for g in range(ng):
            self.group_blk0[g] = b
            for c in range(nch):
                self.slot_base[g, c] = s
                self.blk_base[g, c] = b
                off = 0
                for tl in range(tpg):
                    t = g * tpg + tl
                    self.sub_off[t, c] = off
                    off += cap[t, c]
                self.region_nidx[g, c] = off
                s += off
                b += off // P
        self.total_slots = s
        self.total_blocks = b
        self.group_blocks = [
            int(sum(self.region_nidx[g]) // P) for g in range(ng)
        ]
        # per (g, tl): ordered list of global block ids (for start/stop flags)
        self.tile_blocks = {}
        for g in range(ng):
            for tl in range(self.tpg):
                t = g * self.tpg + tl
                blks = []
                for c in range(nch):
                    b0 = self.blk_base[g, c] + self.sub_off[t, c] // P
                    blks += list(range(b0, b0 + cap[t, c] // P))
                self.tile_blocks[(g, tl)] = blks
        # idx array column offsets (global, in units of 16 slots)
        self.idx_off = np.zeros((ng, nch), np.int64)
        w = 0
        for g in range(ng):
            for c in range(nch):
                self.idx_off[g, c] = w
                w += self.region_nidx[g, c] // 16
        self.idx_w = w


def _sched_caps(sched, per_core_tc_counts):
    """per_core_tc_counts: list of [ntp, nch] arrays -> set caps."""
    mx = np.maximum.reduce(per_core_tc_counts)
    sched.cap = (-(-mx // P) * P).astype(np.int64)
    sched.finalize()


def _fill_spmm(sched, rows_t, cols_c, col_idx, rowloc, vals):
    """Place one core's edges into the schedule's slot space.

    rows_t: tile id per edge; cols_c: chunk id; col_idx: int16 local col;
    rowloc: row-in-tile (0..127); vals: edge values (float32).
    Edges are sorted by column within each (tile, chunk) so the gather's HBM
    addresses ascend (DRAM locality).
    Returns (idx_arr [128, idx_w] i16, rl [128, B] bf16, val_w [128, B] f32,
             counts [ng*nch] i32).
    """
    ntp, tpg, ng, nch = sched.ntp, sched.tpg, sched.ng, sched.nch
    g_e = rows_t // tpg
    tl_e = rows_t % tpg
    bid = (g_e * nch + cols_c) * tpg + tl_e
    order = np.lexsort((col_idx, bid))
    bid_s = bid[order]
    counts = np.bincount(bid_s, minlength=ng * nch * tpg)
    starts = np.concatenate([[0], np.cumsum(counts)[:-1]])
    rank = np.arange(len(bid_s)) - starts[bid_s]
    t_s = rows_t[order]
    c_s = cols_c[order]
    g_s = g_e[order]
    slot = (
        sched.slot_base[g_s, c_s]
        + sched.sub_off[t_s, c_s]
        + rank
    )
    ns = sched.total_slots
    idx_flat = np.zeros(ns, np.int32)
    rl_flat = np.full(ns, -1.0, np.float32)
    val_flat = np.zeros(ns, np.float32)
    idx_flat[slot] = col_idx[order]
    rl_flat[slot] = rowloc[order]
    val_flat[slot] = vals[order]
    # pad slots keep idx 0 (real harmless gathers) so every slot is always
    # written -- avoids NaN garbage flowing into the matmul.
    call_counts = np.zeros(ng * nch, np.int32)
    # wrap
    idx_arr = np.empty((P, sched.idx_w), np.int16)
    for g in range(ng):
        for c in range(nch):
            n = sched.region_nidx[g, c]
            if n == 0:
                continue
            s0 = sched.slot_base[g, c]
            w0 = sched.idx_off[g, c]
            idx_arr[:, w0 : w0 + n // 16] = _wrap_idx(
                idx_flat[s0 : s0 + n].astype(np.int16)
            )
    # host-precomputed one-hot routing matrices, fp8 (exact for 0/1):
    # oh[e, b*128 + r] = 1 iff edge in slot (b, e) targets row r of its tile.
    # Streamed from HBM instead of generated on DVE (is_equal runs at the
    # errata-limited 1x rate and was a ~1ms bottleneck).
    FP8 = mybir.dt.np(mybir.dt.float8e4)
    rl_i = rl_flat.reshape(sched.total_blocks, P).T.astype(np.int32)  # [P, B]
    oh = (rl_i[:, :, None] == np.arange(P, dtype=np.int32)[None, None, :])
    oh = np.ascontiguousarray(oh.astype(FP8).reshape(P, sched.total_blocks * P))
    val_w = np.ascontiguousarray(val_flat.reshape(sched.total_blocks, P).T)
    return idx_arr, oh, val_w, call_counts


def _prep(cfg, inputs):
    """All host-side preprocessing. Returns (plan, in_maps, out_meta)."""
    nc_, d = cfg.nc, cfg.d
    user_emb = np.asarray(inputs["user_emb"], np.float32)
    item_emb = np.asarray(inputs["item_emb"], np.float32)
    s_rows = np.asarray(inputs["social_rows"], np.int64)
    s_cols = np.asarray(inputs["social_cols"], np.int64)
    s_vals = np.asarray(inputs["social_vals"], np.float32)
    i_rows = np.asarray(inputs["info_rows"], np.int64)
    i_cols = np.asarray(inputs["info_cols"], np.int64)
    i_vals = np.asarray(inputs["info_vals"], np.float32)
    uids = np.asarray(inputs["user_ids"], np.int64)
    iids = np.asarray(inputs["item_ids"], np.int64)
    eps = 1e-8

    ones = bool(np.all(s_vals == 1.0) and np.all(i_vals == 1.0))

    deg_soc = np.bincount(s_rows, weights=s_vals, minlength=cfg.n_user)
    deg_info = np.bincount(i_rows, weights=i_vals, minlength=cfg.n_user)
    inv_soc = (1.0 / (deg_soc.astype(np.float32) + eps)).astype(np.float32)
    inv_info = (1.0 / (deg_info.astype(np.float32) + eps)).astype(np.float32)

    # padded bf16 tables: 256B row stride, payload in cols [:64]
    user_pad = np.zeros((cfg.n_user, 2 * d), BF16)
    user_pad[:, :d] = user_emb
    item_pad = np.zeros((cfg.n_item, 2 * d), BF16)
    item_pad[:, :d] = item_emb

    # batch users
    uniq = np.unique(uids)
    owner = uniq // cfg.rpc
    bu = [uniq[owner == c] for c in range(nc_)]
    ubmax = max(len(b) for b in bu)
    ubt = -(-ubmax // P)
    ng2 = max(1, -(-ubt // cfg.tpg2))
    ubt_p = ng2 * cfg.tpg2
    ubp = ubt_p * P

    # --- L1 schedule ---
    s1 = SpmmSched(cfg.t1p, cfg.tpg1, cfg.nch_u)
    order = np.argsort(s_rows, kind="stable")
    sr, sc, sv = s_rows[order], s_cols[order], s_vals[order]
    bounds = np.searchsorted(sr, [c * cfg.rpc for c in range(nc_ + 1)])
    core_l1 = []
    tc_counts = []
    for c in range(nc_):
        lo, hi = bounds[c], bounds[c + 1]
        lr = sr[lo:hi] - c * cfg.rpc
        col = sc[lo:hi]
        t = lr // P
        ch = col // cfg.ch_u
        core_l1.append((t, ch, (col - ch * cfg.ch_u), lr % P, sv[lo:hi]))
        m = np.zeros((cfg.t1p, cfg.nch_u), np.int64)
        np.add.at(m, (t, ch), 1)
        tc_counts.append(m)
    _sched_caps(s1, tc_counts)

    # --- L2 & info: batch-row-restricted ---
    slot_of = np.full(cfg.n_user, -1, np.int64)
    for c in range(nc_):
        slot_of[bu[c]] = np.arange(len(bu[c]))
    in_batch = slot_of >= 0

    def batch_edges(rows, cols, vals):
        m = in_batch[rows]
        r, co, v = rows[m], cols[m], vals[m]
        core = r // cfg.rpc
        return r, co, v, core

    s2 = SpmmSched(ubt_p, cfg.tpg2, cfg.nch_h)
    si = SpmmSched(ubt_p, cfg.tpg2, cfg.nch_i)

    def h1_flat(col):
        own = col // cfg.rpc
        lr = col - own * cfg.rpc
        k = lr // cfg.cr
        off = lr - k * cfg.cr
        return k * (nc_ * cfg.cr) + own * cfg.cr + off

    r2, c2, v2, core2 = batch_edges(sr, sc, sv)
    f2 = h1_flat(c2)
    ri, ci, vi, corei = batch_edges(i_rows, i_cols, i_vals)

    core_l2, core_in = [], []
    tc2, tci = [], []
    for c in range(nc_):
        m = core2 == c
        sl = slot_of[r2[m]]
        t = sl // P
        ch = f2[m] // cfg.ch_h
        core_l2.append((t, ch, f2[m] - ch * cfg.ch_h, sl % P, v2[m]))
        a = np.zeros((ubt_p, cfg.nch_h), np.int64)
        np.add.at(a, (t, ch), 1)
        tc2.append(a)
        m = corei == c
        sl = slot_of[ri[m]]
        t = sl // P
        ch = ci[m] // cfg.ch_i
        core_in.append((t, ch, ci[m] - ch * cfg.ch_i, sl % P, vi[m]))
        a = np.zeros((ubt_p, cfg.nch_i), np.int64)
        np.add.at(a, (t, ch), 1)
        tci.append(a)
    _sched_caps(s2, tc2)
    _sched_caps(si, tci)

    # --- final pairs ---
    pcore = uids // cfg.rpc
    pch = iids // cfg.ch_i
    fcap = np.zeros(cfg.nch_i, np.int64)
    per_core_pairs = []
    for c in range(nc_):
        m = np.nonzero(pcore == c)[0]
        o = m[np.lexsort((iids[m], pch[m]))]
        per_core_pairs.append(o)
        cnts = np.bincount(pch[o], minlength=cfg.nch_i)
        fcap = np.maximum(fcap, cnts)
    fcap = -(-fcap // P) * P
    fcap = np.maximum(fcap, P)
    fbase = np.concatenate([[0], np.cumsum(fcap)])
    ftot = int(fbase[-1])

    plan = dict(
        cfg=cfg, s1=s1, s2=s2, si=si, ubt_p=ubt_p, ubp=ubp, ng2=ng2,
        fcap=fcap, fbase=fbase, ftot=ftot, ones=ones,
    )

    in_maps = []
    out_meta = []  # per core: (pair_js, slots)
    for c in range(nc_):
        t, ch, cidx, rl, v = core_l1[c]
        l1_idx, l1_oh, l1_val, cnt1 = _fill_spmm(s1, t, ch, cidx, rl, v)
        t, ch, cidx, rl, v = core_l2[c]
        l2_idx, l2_oh, l2_val, cnt2 = _fill_spmm(s2, t, ch, cidx, rl, v)
        t, ch, cidx, rl, v = core_in[c]
        in_idx, in_oh, in_val, cnti = _fill_spmm(si, t, ch, cidx, rl, v)

        # u0 shard
        u0s = np.zeros((cfg.shard_rows, d), np.float32)
        nrow = min(cfg.rpc, cfg.n_user - c * cfg.rpc)
        u0s[:nrow] = user_emb[c * cfg.rpc : c * cfg.rpc + nrow]

        # invdeg arrays
        ist = np.zeros((P, cfg.t1p), np.float32)
        rows = c * cfg.rpc + np.arange(nrow)
        ist[np.arange(nrow) % P, np.arange(nrow) // P] = inv_soc[rows]
        isb = np.zeros((P, ubt_p), np.float32)
        iib = np.zeros((P, ubt_p), np.float32)
        nb = len(bu[c])
        isb[np.arange(nb) % P, np.arange(nb) // P] = inv_soc[bu[c]]
        iib[np.arange(nb) % P, np.arange(nb) // P] = inv_info[bu[c]]

        # h1 batch gather idx (local shard rows); pads gather row 0
        h1b = np.zeros(ubp, np.int16)
        h1b[:nb] = (bu[c] - c * cfg.rpc).astype(np.int16)

        # final pairs
        o = per_core_pairs[c]
        pu = np.zeros(ftot, np.int16)
        pi = np.zeros(ftot, np.int16)
        slots = np.empty(len(o), np.int64)
        pos = 0
        for chn in range(cfg.nch_i):
            sel = o[pch[o] == chn]
            k = len(sel)
            s0 = fbase[chn]
            pu[s0 : s0 + k] = slot_of[uids[sel]].astype(np.int16)
            pi[s0 : s0 + k] = (iids[sel] - chn * cfg.ch_i).astype(np.int16)
            slots[pos : pos + k] = s0 + np.arange(k)
            pos += k
        out_meta.append((o, slots))

        m = {
            "user_pad": user_pad,
            "item_pad": item_pad,
            "item_emb": item_emb,
            "u0s": u0s,
            "l1_idx": l1_idx, "l1_oh": l1_oh,
            "l2_idx": l2_idx, "l2_oh": l2_oh,
            "in_idx": in_idx, "in_oh": in_oh,
            "ist": ist, "isb": isb, "iib": iib,
            "h1b_idx": _wrap_idx(h1b),
            "pu_idx": _wrap_idx(pu), "pi_idx": _wrap_idx(pi),
        }
        if not ones:
            m["l1_val"] = l1_val
            m["l2_val"] = l2_val
            m["in_val"] = in_val
        in_maps.append(m)
    return plan, in_maps, out_meta


def _build_program(plan):
    cfg = plan["cfg"]
    s1, s2, si = plan["s1"], plan["s2"], plan["si"]
    ubt_p, ubp, ng2 = plan["ubt_p"], plan["ubp"], plan["ng2"]
    fcap, fbase, ftot = plan["fcap"], plan["fbase"], plan["ftot"]
    ones = plan["ones"]
    d = cfg.d
    nc_ = cfg.nc
    f32 = mybir.dt.float32
    bf = mybir.dt.bfloat16

    # 4 SWDGE queues: each runs on its own GPSIMD DSP core pair, so gathers
    # on different queues generate descriptors concurrently.
    nc = bacc.Bacc("TRN2", debug=False, num_devices=nc_, num_swdge_queues=4)
    qrr = [0]

    def next_q():
        qrr[0] = (qrr[0] + 1) % 4
        return qrr[0]

    t_user = nc.dram_tensor("user_pad", [cfg.n_user, 2 * d], bf, kind="ExternalInput")
    t_itemp = nc.dram_tensor("item_pad", [cfg.n_item, 2 * d], bf, kind="ExternalInput")
    t_item = nc.dram_tensor("item_emb", [cfg.n_item, d], f32, kind="ExternalInput")
    t_u0s = nc.dram_tensor("u0s", [cfg.shard_rows, d], f32, kind="ExternalInput")
    fp8 = mybir.dt.float8e4
    t_l1i = nc.dram_tensor("l1_idx", [P, s1.idx_w], mybir.dt.int16, kind="ExternalInput")
    t_l1o = nc.dram_tensor("l1_oh", [P, s1.total_blocks * P], fp8, kind="ExternalInput")
    t_l2i = nc.dram_tensor("l2_idx", [P, s2.idx_w], mybir.dt.int16, kind="ExternalInput")
    t_l2o = nc.dram_tensor("l2_oh", [P, s2.total_blocks * P], fp8, kind="ExternalInput")
    t_ini = nc.dram_tensor("in_idx", [P, si.idx_w], mybir.dt.int16, kind="ExternalInput")
    t_ino = nc.dram_tensor("in_oh", [P, si.total_blocks * P], fp8, kind="ExternalInput")
    t_ist = nc.dram_tensor("ist", [P, cfg.t1p], f32, kind="ExternalInput")
    t_isb = nc.dram_tensor("isb", [P, ubt_p], f32, kind="ExternalInput")
    t_iib = nc.dram_tensor("iib", [P, ubt_p], f32, kind="ExternalInput")
    t_h1bi = nc.dram_tensor("h1b_idx", [P, ubp // 16], mybir.dt.int16, kind="ExternalInput")
    t_pui = nc.dram_tensor("pu_idx", [P, ftot // 16], mybir.dt.int16, kind="ExternalInput")
    t_pii = nc.dram_tensor("pi_idx", [P, ftot // 16], mybir.dt.int16, kind="ExternalInput")
    t_scores = nc.dram_tensor("scores", [P, ftot // P], f32, kind="ExternalOutput")
    t_vals = {}
    if not ones:
        t_vals["l1"] = nc.dram_tensor("l1_val", [P, s1.total_blocks], f32, kind="ExternalInput")
        t_vals["l2"] = nc.dram_tensor("l2_val", [P, s2.total_blocks], f32, kind="ExternalInput")
        t_vals["in"] = nc.dram_tensor("in_val", [P, si.total_blocks], f32, kind="ExternalInput")

    with tile.TileContext(nc) as tc:
        with (
            tc.tile_pool(name="const", bufs=1) as cp,
            tc.tile_pool(name="persist", bufs=1) as pp,
            tc.tile_pool(name="idx", bufs=10) as idxp,
            tc.tile_pool(name="msgs", bufs=8) as msgp,
            tc.tile_pool(name="msgf", bufs=2) as mfp,
            tc.tile_pool(name="oh", bufs=6) as ohp,
            tc.tile_pool(name="rl", bufs=6) as rlp,
            tc.tile_pool(name="u0t", bufs=2) as u0p,
            tc.tile_pool(name="hrow", bufs=4) as hp,
            tc.tile_pool(name="psacc", bufs=4, space="PSUM") as pap,
            tc.tile_pool(name="psinfo", bufs=2, space="PSUM") as ipap,
            tc.tile_pool(name="psl2", bufs=2, space="PSUM") as l2ap,
            tc.tile_pool(name="dram", bufs=1, space="DRAM") as dram,
        ):
            # ---- constants / persistent ----
            ist_t = pp.tile([P, cfg.t1p], f32, tag="ist")
            nc.sync.dma_start(ist_t[:], t_ist.ap())
            isb_t = pp.tile([P, ubt_p], f32, tag="isb")
            nc.sync.dma_start(isb_t[:], t_isb.ap())
            iib_t = pp.tile([P, ubt_p], f32, tag="iib")
            nc.sync.dma_start(iib_t[:], t_iib.ap())
            t3R = pp.tile([P, ubt_p, d], f32, tag="t3R")
            h1b_t = pp.tile([P, ubt_p, d], f32, tag="h1b")
            nc.vector.memzero(h1b_t[:])

            # internal DRAM
            h1ag = [
                dram.tile([cfg.cr, d], bf, tag=f"h1ag{k}", name=f"h1ag{k}")
                for k in range(cfg.agc)
            ]
            h1fb = [
                dram.tile([nc_ * cfg.cr, d], bf, tag=f"h1fb{k}",
                          name=f"h1fb{k}")
                for k in range(cfg.agc)
            ]
            h1_full = dram.tile([cfg.flat_h1, 2 * d], bf, tag="h1full")
            h1_shard = dram.tile([cfg.shard_rows, d], f32, tag="h1shard")
            fu_tab = dram.tile([ubp, d], f32, tag="futab")

            def load_rl(sched, g, val_t):
                gb0 = int(sched.group_blk0[g])
                gblocks = sched.group_blocks[g]
                vw_t = None
                if val_t is not None:
                    vw_t = rlp.tile([P, gblocks], f32, tag="vw")
                    nc.sync.dma_start(vw_t[:], val_t.ap()[:, gb0 : gb0 + gblocks])
                first = {tl: sched.tile_blocks[(g, tl)][0] for tl in range(sched.tpg)
                         if sched.tile_blocks[(g, tl)]}
                last = {tl: sched.tile_blocks[(g, tl)][-1] for tl in range(sched.tpg)
                        if sched.tile_blocks[(g, tl)]}
                return vw_t, gb0, first, last

            def spmm_region(sched, g, c, grp, t_idx, t_oh, table_aps, psums,
                            bf_gather):
                """Emit gathers/onehot-load/matmuls for one (group, chunk)
                region.

                psums[tl]: PSUM [128, d] accumulator slice for each tile.
                bf_gather: gather 128B bf16 payloads from a padded table
                (table_aps are [:, :64] views of 256B-stride bf16 tables);
                else classic 256B f32 dma_gather.
                """
                vw_t, gb0, first, last = grp
                nidx = int(sched.region_nidx[g, c])
                if nidx == 0:
                    return
                rb = nidx // P
                w0 = int(sched.idx_off[g, c])
                it = idxp.tile([P, nidx // 16], mybir.dt.int16, tag="idx")
                nc.sync.dma_start(it[:], t_idx.ap()[:, w0 : w0 + nidx // 16])
                # prefetch the fp8 one-hot stream so it overlaps the gathers
                b0 = int(sched.blk_base[g, c])
                oh_t = ohp.tile([P, rb * P], mybir.dt.float8e4, tag="oh")
                nc.sync.dma_start(
                    oh_t[:], t_oh.ap()[:, b0 * P : (b0 + rb) * P]
                )
                if bf_gather:
                    m_t = msgp.tile([P, rb, d], bf, tag="msgs")
                    for s0 in range(0, nidx, GSUB):
                        n = min(GSUB, nidx - s0)
                        _gather_raw(
                            nc,
                            m_t[:, s0 // P : (s0 + n) // P, :],
                            table_aps[c],
                            it[:, s0 // 16 : (s0 + n) // 16],
                            n, d, 2 * d, next_q(),
                        )
                else:
                    m_t = mfp.tile([P, rb, d], f32, tag="msgf")
                    for s0 in range(0, nidx, GSUB):
                        n = min(GSUB, nidx - s0)
                        nc.gpsimd.dma_gather(
                            m_t[:, s0 // P : (s0 + n) // P, :],
                            table_aps[c],
                            it[:, s0 // 16 : (s0 + n) // 16],
                            n, n, d, single_packet=False,
                            queue_num=next_q(),
                        )
                if vw_t is not None:
                    rboff0 = int(sched.blk_base[g, c]) - gb0
                    nc.vector.tensor_tensor(
                        out=m_t[:],
                        in0=m_t[:],
                        in1=vw_t[:, rboff0 : rboff0 + rb]
                        .unsqueeze(2)
                        .to_broadcast([P, rb, d]),
                        op=mybir.AluOpType.mult,
                    )
                mb_src = m_t
                if not bf_gather:
                    mb_t = msgp.tile([P, rb, d], bf, tag="msgs")
                    nc.scalar.copy(mb_t[:], m_t[:])
                    mb_src = mb_t
                for j in range(rb):
                    gblk = b0 + j
                    soff = j * P
                    tl = 0
                    for tt in range(sched.tpg):
                        t_ = g * sched.tpg + tt
                        if (sched.sub_off[t_, c] <= soff
                                < sched.sub_off[t_, c] + sched.cap[t_, c]):
                            tl = tt
                            break
                    nc.tensor.matmul(
                        psums[tl],
                        lhsT=oh_t[:, j * P : (j + 1) * P],
                        rhs=mb_src[:, j, :],
                        start=(gblk == first[tl]),
                        stop=(gblk == last[tl]),
                    )

            def psum_packs(tpg):
                # row-major [128, tpg*d] f32 accumulator pack (<=1 bank)
                assert tpg * d * 4 <= 2048
                return pap.tile([P, tpg * d], f32, tag="acc", name="accpk")

            def spmm_group(sched, g, t_idx, t_oh, table_aps, val_t, psums,
                           bf_gather):
                if sched.group_blocks[g] == 0:
                    return
                grp = load_rl(sched, g, val_t)
                for c in range(sched.nch):
                    spmm_region(sched, g, c, grp, t_idx, t_oh, table_aps,
                                psums, bf_gather)

            # ================= L1 (with INFO interleaved) =================
            user_chunks = [
                t_user.ap()[c * cfg.ch_u : min((c + 1) * cfg.ch_u, cfg.n_user), :d]
                for c in range(cfg.nch_u)
            ]
            itemp_chunks = [
                t_itemp.ap()[c * cfg.ch_i : min((c + 1) * cfg.ch_i, cfg.n_item), :d]
                for c in range(cfg.nch_i)
            ]

            def info_group(gi):
                # INFO SpMM is independent of the AllGather chain; interleave
                # its groups into the L1 loop to fill gather-queue idle time.
                pack = ipap.tile([P, si.tpg * d], f32, tag="iacc")
                psums = [pack[:, tl * d : (tl + 1) * d] for tl in range(si.tpg)]
                spmm_group(si, gi, t_ini, t_ino, itemp_chunks,
                           t_vals.get("in"), psums, bf_gather=True)
                for tl in range(si.tpg):
                    T = gi * si.tpg + tl
                    nc.vector.tensor_copy(t3R[:, T, :], psums[tl])

            # L2 setup: regions interleave into the L1 loop as soon as the
            # AllGather chunks behind each h1 chunk land (chunk c needs AG
            # k<=ceil(((c+1)*ch_h)/(nc*cr))-1; schedule with 1-group margin).
            h1_chunks = [
                h1_full[c * cfg.ch_h : min((c + 1) * cfg.ch_h, cfg.flat_h1), :d]
                for c in range(cfg.nch_h)
            ]
            # two groups share one bank-sized [128, 512] f32 pack
            assert s2.ng % 2 == 0 and 2 * s2.tpg * d * 4 <= 2048
            l2_packs = [
                l2ap.tile([P, 2 * s2.tpg * d], f32, tag="l2acc", name="l2pk")
                for _ in range(s2.ng // 2)
            ]
            l2_psums = [
                [
                    l2_packs[g // 2][
                        :, ((g % 2) * s2.tpg + tl) * d
                        : ((g % 2) * s2.tpg + tl + 1) * d
                    ]
                    for tl in range(s2.tpg)
                ]
                for g in range(s2.ng)
            ]
            l2_grps = [
                load_rl(s2, g, t_vals.get("l2")) for g in range(s2.ng)
            ]

            def l2_phase(c):
                for g in range(s2.ng):
                    if s2.group_blocks[g] == 0:
                        continue
                    spmm_region(s2, g, c, l2_grps[g], t_l2i, t_l2o, h1_chunks,
                                l2_psums[g], bf_gather=True)

            info_after = {2: 0, 4: 1, 8: 2, 11: 3} if s1.ng >= 12 else {}
            l2_after = {}
            info_done = set()
            l2_done = set()
            for g in range(s1.ng):
                pack = psum_packs(s1.tpg)
                psums = [pack[:, tl * d : (tl + 1) * d] for tl in range(s1.tpg)]
                spmm_group(
                    s1, g, t_l1i, t_l1o, user_chunks,
                    t_vals.get("l1"), psums, bf_gather=True,
                )
                u0_t = u0p.tile([P, s1.tpg, d], f32, tag="u0t")
                r0 = g * s1.tpg * P
                nc.sync.dma_start(
                    u0_t[:],
                    t_u0s.ap()[r0 : r0 + s1.tpg * P, :].rearrange(
                        "(t p) d -> p t d", p=P
                    ),
                )
                for tl in range(s1.tpg):
                    gt = g * s1.tpg + tl
                    h1_t = hp.tile([P, d], f32, tag="hrow")
                    nc.vector.scalar_tensor_tensor(
                        out=h1_t[:],
                        in0=psums[tl],
                        scalar=ist_t[:, gt : gt + 1],
                        in1=u0_t[:, tl, :],
                        op0=mybir.AluOpType.mult,
                        op1=mybir.AluOpType.add,
                    )
                    k = g // cfg.gpa
                    lrow = ((g % cfg.gpa) * s1.tpg + tl) * P
                    h1_b16 = hp.tile([P, d], bf, tag="hrowb")
                    nc.scalar.copy(h1_b16[:], h1_t[:])
                    nc.sync.dma_start(
                        h1ag[k][lrow : lrow + P, :], h1_b16[:]
                    )
                    nc.sync.dma_start(
                        h1_shard[gt * P : (gt + 1) * P, :], h1_t[:]
                    )
                if (g + 1) % cfg.gpa == 0:
                    k = g // cfg.gpa
                    o0 = k * nc_ * cfg.cr
                    nc.gpsimd.collective_compute(
                        "AllGather",
                        mybir.AluOpType.bypass,
                        replica_groups=[list(range(nc_))],
                        ins=[h1ag[k][:].opt()],
                        outs=[h1fb[k][:].opt()],
                    )
                    # expand AG output into the 256B-stride padded bf16
                    # gather table, pipelined per AG chunk
                    nc.gpsimd.dma_start(
                        h1_full[o0 : o0 + nc_ * cfg.cr, :d], h1fb[k][:]
                    )
                if g in info_after:
                    info_group(info_after[g])
                    info_done.add(info_after[g])
                if g in l2_after:
                    l2_phase(l2_after[g])
                    l2_done.add(l2_after[g])

            # h1 batch rows gather (from own shard)
            h1bi_t = pp.tile([P, ubp // 16], mybir.dt.int16, tag="h1bidx")
            nc.sync.dma_start(h1bi_t[:], t_h1bi.ap())
            for s0 in range(0, ubp, GSUB):
                n = min(GSUB, ubp - s0)
                nc.gpsimd.dma_gather(
                    h1b_t[:, s0 // P : (s0 + n) // P, :],
                    h1_shard[:],
                    h1bi_t[:, s0 // 16 : (s0 + n) // 16],
                    n, n, d, single_packet=False,
                    queue_num=next_q(),
                )

            # ================= INFO (groups not yet interleaved) ============
            for gi in range(si.ng):
                if gi not in info_done:
                    info_group(gi)

            # ================= L2 (remaining chunk phases + drains) =========
            for c in range(s2.nch):
                if c not in l2_done:
                    l2_phase(c)
            for g in range(s2.ng):
                psums = l2_psums[g]
                for tl in range(s2.tpg):
                    T = g * s2.tpg + tl
                    x1 = hp.tile([P, d], f32, tag="hrow")
                    nc.vector.tensor_scalar_mul(
                        x1[:], psums[tl], isb_t[:, T : T + 1]
                    )
                    x2 = hp.tile([P, d], f32, tag="hrow")
                    nc.vector.scalar_tensor_tensor(
                        out=x2[:], in0=t3R[:, T, :],
                        scalar=iib_t[:, T : T + 1], in1=x1[:],
                        op0=mybir.AluOpType.mult, op1=mybir.AluOpType.add,
                    )
                    fu = hp.tile([P, d], f32, tag="hrow")
                    nc.vector.scalar_tensor_tensor(
                        out=fu[:], in0=h1b_t[:, T, :], scalar=2.0, in1=x2[:],
                        op0=mybir.AluOpType.mult, op1=mybir.AluOpType.add,
                    )
                    nc.sync.dma_start(fu_tab[T * P : (T + 1) * P, :], fu[:])

            # ================= FINAL =================
            item_chunks_f32 = [
                t_item.ap()[c * cfg.ch_i : min((c + 1) * cfg.ch_i, cfg.n_item), :]
                for c in range(cfg.nch_i)
            ]
            sc_t = pp.tile([P, ftot // P], f32, tag="scores")
            for chn in range(cfg.nch_i):
                n = int(fcap[chn])
                s0 = int(fbase[chn])
                fb = n // P
                iu = idxp.tile([P, n // 16], mybir.dt.int16, tag="idx")
                nc.sync.dma_start(
                    iu[:], t_pui.ap()[:, s0 // 16 : (s0 + n) // 16]
                )
                ii = idxp.tile([P, n // 16], mybir.dt.int16, tag="idx")
                nc.sync.dma_start(
                    ii[:], t_pii.ap()[:, s0 // 16 : (s0 + n) // 16]
                )
                u_t = mfp.tile([P, fb, d], f32, tag="msgf")
                v_t = mfp.tile([P, fb, d], f32, tag="msgf")
                for q0 in range(0, n, GSUB):
                    nq = min(GSUB, n - q0)
                    nc.gpsimd.dma_gather(
                        u_t[:, q0 // P : (q0 + nq) // P, :], fu_tab[:],
                        iu[:, q0 // 16 : (q0 + nq) // 16], nq, nq, d,
                        single_packet=False, queue_num=next_q(),
                    )
                    nc.gpsimd.dma_gather(
                        v_t[:, q0 // P : (q0 + nq) // P, :], item_chunks_f32[chn],
                        ii[:, q0 // 16 : (q0 + nq) // 16], nq, nq, d,
                        single_packet=False, queue_num=next_q(),
                    )
                pr = mfp.tile([P, fb, d], f32, tag="prod")
                nc.vector.tensor_mul(pr[:], u_t[:], v_t[:])
                dot = hp.tile([P, fb], f32, tag="dot")
                nc.vector.tensor_reduce(
                    dot[:], pr[:], axis=mybir.AxisListType.X,
                    op=mybir.AluOpType.add,
                )
                nc.scalar.activation(
                    sc_t[:, s0 // P : (s0 + n) // P], dot[:],
                    mybir.ActivationFunctionType.Sigmoid, scale=2.0,
                )
            nc.sync.dma_start(t_scores.ap(), sc_t[:])

    nc.compile()
    return nc


_CACHE = {}


def _run(cfg, inputs, trace=False):
    import time as _time

    _t = _time.time()
    plan, in_maps, out_meta = _prep(cfg, inputs)
    print(f"[kernel] prep: {_time.time()-_t:.1f}s", flush=True)
    _t = _time.time()
    key = (
        cfg.n_user, plan["s1"].total_slots, plan["s2"].total_slots,
        plan["si"].total_slots, plan["ubt_p"], plan["ftot"], plan["ones"],
    )
    if key not in _CACHE:
        _CACHE[key] = _build_program(plan)
        print(f"[kernel] build+compile: {_time.time()-_t:.1f}s", flush=True)
    nc = _CACHE[key]
    _t = _time.time()
    kw = {}
    if trace:
        # single-core NTFF (SPMD cores are balanced); exec_time_ns comes back
        kw = dict(trace=True, trace_cores=[0])
    res = run_bass_kernel_spmd(
        nc, in_maps, core_ids=list(range(cfg.nc)), **kw
    )
    print(f"[kernel] run: {_time.time()-_t:.1f}s", flush=True)
    out = np.zeros(len(inputs["user_ids"]), np.float32)
    for c in range(cfg.nc):
        js, slots = out_meta[c]
        sc = res.results[c]["scores"]
        out[js] = sc[slots % P, slots // P]
    return out, res


def kernel(**inputs):
    out, _ = _run(REAL, inputs, trace=bool(os.environ.get("KERNEL_TRACE")))
    return out
